# revision 20
# baseline (speedup 1.0000x reference)
"""Multi-head attention (B=2, S=4096, D=512, H=8) on 8 TRN2 NeuronCores.

Sharding: batch x head-pair (tensor parallel). Core c handles batch
b=c//4 and heads {2p, 2p+1} with p=c%4, over the FULL 4096-token
sequence. Q/K/V/O projections are sliced along the head dimension
(each core projects only its 128 dims), eliminating the redundant
full K/V projection of token-sharding. Each core emits a PARTIAL
output (its heads' contribution through w_o); the host sums the four
partials per batch. V/O biases are folded on the host into a per-core
additive vector (boeff); Q/K biases are added on-device via rank-1
matmuls.

Attention is flash-style with scores kept transposed [tk, tq]. The
two heads' score matmuls (contraction 64) are issued back-to-back at
PE row groups 0/64, so they run CONCURRENTLY in the systolic array
(row tiling) - scores cost half of token-sharding. Softmax skips the
max-subtraction (scores ~ N(0,1)) and the denominator comes from a
ones column appended to V, so softmax is exactly one ACT pass per
score block. The kernel is ACT(exp)-bound: 256 activations of
[128,1024] are the critical path; the schedule keeps ACT fed by
emitting next-group scores before current-group PV and interleaving
all projection work into early-group slack.
"""

import numpy as np
import ml_dtypes

B, S, D = 2, 4096, 512
H, DK = 8, 64
N_CORES = 8
PD = 128  # dims per core (2 heads x 64)
NTQ = 8  # tq tiles of 512
NCH = 32  # tk chunks of 128

_PROGRAM = None


def _build_program():
    from contextlib import ExitStack

    import concourse.mybir as mybir
    import concourse.tile as tile
    from concourse import bacc

    bf = mybir.dt.bfloat16
    f32 = mybir.dt.float32
    Exp = mybir.ActivationFunctionType.Exp

    nc = bacc.Bacc(None)

    qT = nc.declare_dram_parameter("qT", [D, S], bf, isOutput=False)
    kT = nc.declare_dram_parameter("kT", [D, S], bf, isOutput=False)
    vT = nc.declare_dram_parameter("vT", [D, S], bf, isOutput=False)
    wqT = nc.declare_dram_parameter("wqT", [D, PD], bf, isOutput=False)
    wkT = nc.declare_dram_parameter("wkT", [D, PD], bf, isOutput=False)
    wvT = nc.declare_dram_parameter("wvT", [D, PD], bf, isOutput=False)
    woT = nc.declare_dram_parameter("woT", [PD, D], bf, isOutput=False)
    bq = nc.declare_dram_parameter("bq", [1, PD], bf, isOutput=False)
    bk = nc.declare_dram_parameter("bk", [1, PD], bf, isOutput=False)
    out_p = nc.declare_dram_parameter("out", [S, D], bf, isOutput=True)

    with tile.TileContext(nc) as tc, ExitStack() as ctx:
        wpool = ctx.enter_context(tc.tile_pool(name="w", bufs=1))
        kstream = ctx.enter_context(tc.tile_pool(name="kstream", bufs=2))
        qstream = ctx.enter_context(tc.tile_pool(name="qstream", bufs=2))
        vstream = ctx.enter_context(tc.tile_pool(name="vstream", bufs=2))
        khpool = ctx.enter_context(tc.tile_pool(name="kh", bufs=1))
        qhpool = ctx.enter_context(tc.tile_pool(name="qh", bufs=1))
        vstore = ctx.enter_context(tc.tile_pool(name="vstore", bufs=33))
        ptpool = ctx.enter_context(tc.tile_pool(name="pt", bufs=4))
        opool = ctx.enter_context(tc.tile_pool(name="o", bufs=4))
        wsp = ctx.enter_context(tc.tile_pool(name="ws", bufs=6))
        ostage = ctx.enter_context(tc.tile_pool(name="ostage", bufs=2))
        scorep = ctx.enter_context(tc.tile_pool(name="scorep", bufs=2, space="PSUM"))
        pvp = ctx.enter_context(tc.tile_pool(name="pvp", bufs=2, space="PSUM"))
        projp = ctx.enter_context(tc.tile_pool(name="projp", bufs=2, space="PSUM"))

        dma = nc.sync.dma_start
        MM = nc.tensor.matmul

        # ---- constants (DMA order = sync-queue order = the startup
        # critical path: K-proj inputs first, then Q, then V, the rest) ----
        ones1 = wpool.tile([1, D], bf, tag="ones", name="ones1")
        nc.vector.memset(ones1[:], 1.0)
        # warm the ACT table (exp) during the DMA-heavy prefix
        wrm = wsp.tile([1, 16], f32, tag="denb", name="warm")
        nc.vector.memset(wrm[:], 0.0)
        wrm2 = wsp.tile([1, 16], f32, tag="rrow", name="warm2")
        nc.scalar.activation(out=wrm2[:], in_=wrm[:], func=Exp, scale=1.0)
        # keep the PE busy through the DMA-bound prefix so HAM reaches
        # K=8/8 before (and stays there for) the real matmul stream
        wps = projp.tile([128, 512], f32, tag="proj", name="warm_ps")
        for _ in range(24):
            MM(
                wps[0:1, :],
                ones1[0:1, 0:1],
                ones1[0:1, :],
                start=True,
                stop=True,
                skip_group_check=True,
            )

        def wtiles(param, tagp):
            t = wpool.tile([128, 4, PD], bf, tag=tagp, name=tagp)
            dma(out=t[:], in_=param[:].rearrange("(c p) d -> p c d", p=128))
            return t

        khT = khpool.tile([PD, S], bf, tag="khT", name="khT")
        qhT = qhpool.tile([PD, S], bf, tag="qhT", name="qhT")
        v_store = [None] * NCH  # [128 tok, 2 heads, DK+1]; col 64 = ones

        kraw_t = {}
        qraw_t = {}
        vraw_t = {}

        def dma_kraw(t):
            kr = kstream.tile([128, 4, 512], bf, tag="kraw", name="kraw")
            dma(
                out=kr[:],
                in_=kT[:, t * 512 : (t + 1) * 512].rearrange(
                    "(c p) t -> p c t", p=128
                ),
            )
            kraw_t[t] = kr

        def dma_qraw(t):
            qr = qstream.tile([128, 4, 512], bf, tag="qraw", name="qraw")
            dma(
                out=qr[:],
                in_=qT[:, t * 512 : (t + 1) * 512].rearrange(
                    "(c p) t -> p c t", p=128
                ),
            )
            qraw_t[t] = qr

        def dma_vraw(r):
            vr = vstream.tile([128, 4, 512], bf, tag="vraw", name="vraw")
            dma(
                out=vr[:],
                in_=vT[:, r * 512 : (r + 1) * 512].rearrange(
                    "(c p) t -> p c t", p=128
                ),
            )
            vraw_t[r] = vr

        def proj_qk(raw, w_t, b_t, dst, t):
            """Project K or Q for token tile t -> dst[:, t*512:(t+1)*512]."""
            ps = projp.tile([128, 512], f32, tag="proj", name="proj_ps")
            for kk in range(4):
                MM(
                    ps[:],
                    w_t[:, kk, :],
                    raw[:, kk, :],
                    start=(kk == 0),
                    stop=False,
                    skip_group_check=True,
                )
            MM(
                ps[:],
                b_t[:],
                ones1[0:1, 0:512],
                start=False,
                stop=True,
                skip_group_check=True,
            )
            nc.vector.tensor_copy(out=dst[:, t * 512 : (t + 1) * 512], in_=ps[:])

        def proj_v_sub(r, sub):
            """Project V tokens (4r+sub)*128.. into v_store[4r+sub].

            Fresh PSUM tile per sub-chunk: sharing one bank across
            sub-chunks makes the DVE copy of chunk n concurrent with PE
            writes of chunk n+1 in the same bank (fatal PSUM collision).
            """
            j = 4 * r + sub
            ps = projp.tile([128, 512], f32, tag="proj", name="vps")
            for kk in range(4):
                MM(
                    ps[:, 0:128],
                    vraw_t[r][:, kk, sub * 128 : (sub + 1) * 128],
                    wv_t[:, kk, :],
                    start=(kk == 0),
                    stop=(kk == 3),
                    skip_group_check=True,
                )
            vs = vstore.tile([128, 2, DK + 1], bf, tag="vs", name="vs")
            v_store[j] = vs
            nc.vector.memset(vs[:, :, DK : DK + 1], 1.0)
            nc.vector.tensor_copy(
                out=vs[:, :, 0:DK],
                in_=ps[:, 0:128].rearrange("p (h c) -> p h c", c=DK),
            )

        def emit_scores(tqt, j):
            sc = scorep.tile([128, 1024], f32, tag="sc", name="sc")
            for h in range(2):
                pb = h * 64
                MM(
                    sc[:, h * 512 : (h + 1) * 512],
                    khT[pb : pb + 64, j * 128 : (j + 1) * 128],
                    qhT[pb : pb + 64, tqt * 512 : (tqt + 1) * 512],
                    start=True,
                    stop=True,
                    skip_group_check=True,
                )
            return sc

        # ---- closure schedule: group index -> list of closures ----
        extra = {}

        def add(g, fn):
            extra.setdefault(g, []).append(fn)

        # K tiles 1-7: dma 4 groups ahead of the matmuls
        for t in range(1, 8):
            add(4 * t - 4, lambda t=t: dma_kraw(t))
            add(
                4 * t - 2,
                lambda t=t: proj_qk(kraw_t[t], wk_t, bk_t, khT, t),
            )
        # V raw streams r=2..7 (r=0,1 in prefix); sub-closure for chunk j
        # runs at group j-4
        for r in range(2, 8):
            add(4 * r - 8, lambda r=r: dma_vraw(r))
        for j in range(4, NCH):
            add(j - 4, lambda r=j // 4, s=j % 4: proj_v_sub(r, s))
        # Q tiles 1-7 projected near the end of the previous tq tile
        for T in range(1, 8):
            add((T - 1) * 32 + 24, lambda T=T: dma_qraw(T))
            add(
                (T - 1) * 32 + 26,
                lambda T=T: proj_qk(qraw_t[T], wq_t, bq_t, qhT, T),
            )

        # ---- tile close + normalization/out-projection ----
        # At tile close: per head, copy the denominator row out of PSUM,
        # reciprocal it on DVE ([1,512], one lane), broadcast across 64
        # partitions on the (idle) GPSIMD engine, and multiply the
        # unnormalized O^T out of PSUM into a normalized bf16 tile. No
        # DMA round-trips; the sync queue stays free for input streaming.
        def emit_close(tqt, pv_tiles):
            o_n = []
            for h in range(2):
                dsb = wsp.tile([1, 512], f32, tag="denb", name="denb")
                nc.vector.tensor_copy(out=dsb[:], in_=pv_tiles[h][64:65, :])
                rrow = wsp.tile([1, 512], f32, tag="rrow", name="rrow")
                nc.vector.reciprocal(out=rrow[:], in_=dsb[:])
                rb = wsp.tile([DK, 512], f32, tag="rb", name="rb")
                nc.gpsimd.partition_broadcast(rb[:], rrow[:])
                on = opool.tile([DK, 512], bf, tag="oh", name="oh")
                nc.vector.tensor_mul(
                    out=on[:], in0=pv_tiles[h][0:64, :], in1=rb[:]
                )
                o_n.append(on)
            return o_n

        def make_close_steps(tqt, o_n):
            steps = []
            for tt in range(4):

                def c3(tt=tt):
                    pa = projp.tile([128, 512], f32, tag="proj", name="pa")
                    for h in range(2):
                        MM(
                            pa[:],
                            o_n[h][:, tt * 128 : (tt + 1) * 128],
                            wo_t[:, h, :],
                            start=(h == 0),
                            stop=(h == 1),
                            skip_group_check=True,
                        )
                    ot = ostage.tile([128, 512], bf, tag="ot", name="ot")
                    nc.vector.tensor_copy(out=ot[:], in_=pa[:])
                    dma(
                        out=out_p[
                            tqt * 512 + tt * 128 : tqt * 512 + (tt + 1) * 128, :
                        ],
                        in_=ot[:],
                    )

                steps.append(c3)
            return steps

        CLOSE_SLOTS = (4, 8, 12, 16)
        pend = {}

        # ---- prefix: interleave input DMAs with the projections that
        # consume them so the first exp fires as early as possible ----
        dma_kraw(0)
        wk_t = wtiles(wkT, "wk")
        bk_t = wpool.tile([1, PD], bf, tag="bk", name="bk_t")
        dma(out=bk_t[:], in_=bk[:])
        dma_qraw(0)
        wq_t = wtiles(wqT, "wq")
        bq_t = wpool.tile([1, PD], bf, tag="bq", name="bq_t")
        dma(out=bq_t[:], in_=bq[:])
        dma_vraw(0)
        wv_t = wtiles(wvT, "wv")
        # [64, 2, D]: wo_t[:, h, :] puts both heads' w_o rows at base
        # partition 0, so out-proj MMs share row group 0 (concurrent
        # row-group accumulation into one PSUM bank races).
        wo_t = wpool.tile([DK, 2, D], bf, tag="wo", name="wo_t")
        dma(out=wo_t[:], in_=woT[:].rearrange("(h p) d -> p h d", p=DK))
        dma_vraw(1)
        proj_qk(kraw_t[0], wk_t, bk_t, khT, 0)
        proj_qk(qraw_t[0], wq_t, bq_t, qhT, 0)
        for sub in range(4):
            proj_v_sub(0, sub)

        # ---- main attention loop ----
        sc_next = emit_scores(0, 0)
        pv_tiles = None
        for g in range(NTQ * NCH):
            tqt, j = divmod(g, NCH)
            if j == 0:
                pv_tiles = [
                    pvp.tile([DK + 1, 512], f32, tag="pv", name=f"pv{_h}")
                    for _h in range(2)
                ]
            sc = sc_next
            pt = ptpool.tile([128, 1024], bf, tag="pt", name="pt")
            nc.scalar.activation(out=pt[:], in_=sc[:], func=Exp, scale=0.125)
            # next group's scores first: nothing else may delay the PE
            # work that feeds ACT
            if g + 1 < NTQ * NCH:
                ntqt, nj = divmod(g + 1, NCH)
                sc_next = emit_scores(ntqt, nj)
            # interleaved work (projections, previous tile's out-proj)
            for fn in extra.get(g, ()):
                fn()
            if tqt >= 1 and j in CLOSE_SLOTS and (tqt - 1) in pend:
                pend[tqt - 1][CLOSE_SLOTS.index(j)]()
            for h in range(2):
                MM(
                    pv_tiles[h][:],
                    v_store[j][:, h, :],
                    pt[:, h * 512 : (h + 1) * 512],
                    start=(j == 0),
                    stop=(j == NCH - 1),
                    skip_group_check=True,
                )
            if j == NCH - 1:
                o_u = emit_close(tqt, pv_tiles)
                pend[tqt] = make_close_steps(tqt, o_u)

        # ---- tail: last tq tile's normalization + out-projection ----
        for fn in pend[NTQ - 1]:
            fn()

    if not nc.is_finalized():
        nc.finalize()
    return nc


def _get_program():
    global _PROGRAM
    if _PROGRAM is None:
        _PROGRAM = _build_program()
    return _PROGRAM


def _prep_inputs(q, k, v, w_q, b_q, w_k, b_k, w_v, b_v, w_o, b_o):
    bf16 = ml_dtypes.bfloat16
    q = np.asarray(q, dtype=np.float32)
    k = np.asarray(k, dtype=np.float32)
    v = np.asarray(v, dtype=np.float32)
    w_q = np.asarray(w_q, np.float32)
    w_k = np.asarray(w_k, np.float32)
    w_v = np.asarray(w_v, np.float32)
    w_o = np.asarray(w_o, np.float32)
    b_q = np.asarray(b_q, np.float32)
    b_k = np.asarray(b_k, np.float32)
    b_v = np.asarray(b_v, np.float32)
    b_o = np.asarray(b_o, np.float32)

    qT = [np.ascontiguousarray(q[b].T).astype(bf16) for b in range(B)]
    kTb = [np.ascontiguousarray(k[b].T).astype(bf16) for b in range(B)]
    vTb = [np.ascontiguousarray(v[b].T).astype(bf16) for b in range(B)]
    wqT = np.ascontiguousarray(w_q.T).astype(bf16)  # [D_in, D_out]
    wkT = np.ascontiguousarray(w_k.T).astype(bf16)
    wvT = np.ascontiguousarray(w_v.T).astype(bf16)
    woT = np.ascontiguousarray(w_o.T)  # [D_in(head dims), D_out] f32

    in_maps = []
    for c in range(N_CORES):
        b, p = divmod(c, 4)
        ds = slice(p * PD, (p + 1) * PD)
        in_maps.append(
            {
                "qT": qT[b],
                "kT": kTb[b],
                "vT": vTb[b],
                "wqT": np.ascontiguousarray(wqT[:, ds]),
                "wkT": np.ascontiguousarray(wkT[:, ds]),
                "wvT": np.ascontiguousarray(wvT[:, ds]),
                "woT": np.ascontiguousarray(woT[ds, :]).astype(bf16),
                "bq": b_q[ds].reshape(1, PD).astype(bf16),
                "bk": b_k[ds].reshape(1, PD).astype(bf16),
            }
        )
    # V/O biases are exact per-token constants: fold on the host instead
    # of spending device matmuls (partials exclude them; added in combine)
    bias_full = (b_v @ woT + b_o).astype(np.float32)
    return in_maps, bias_full


def run_cores(in_maps, trace=False, **kw):
    """Compile+run the SPMD program; returns BassKernelResults."""
    from concourse.bass_utils import run_bass_kernel_spmd

    nc = _get_program()
    return run_bass_kernel_spmd(nc, in_maps, list(range(N_CORES)), trace=trace, **kw)


def combine_outputs(res, bias_full):
    """Sum the per-core partial outputs into the full [B, S, D] result."""
    out = np.zeros((B, S, D), np.float32)
    for c in range(N_CORES):
        b = c // 4
        out[b] += res.results[c]["out"]
    out += bias_full
    return out


def kernel(q, k, v, w_q, b_q, w_k, b_k, w_v, b_v, w_o, b_o):
    in_maps, bias_full = _prep_inputs(
        q, k, v, w_q, b_q, w_k, b_k, w_v, b_v, w_o, b_o
    )
    res = run_cores(in_maps)
    return combine_outputs(res, bias_full)


# revision 29
# speedup vs baseline: 1.1283x; 1.1283x over previous
"""Multi-head attention (B=2, S=4096, D=512, H=8) on 8 TRN2 NeuronCores.

Sharding: batch x head-pair (tensor parallel). Core c handles batch
b=c//4 and heads {2p, 2p+1} with p=c%4, over the FULL 4096-token
sequence. Q/K/V/O projections are sliced along the head dimension
(each core projects only its 128 dims), eliminating the redundant
full K/V projection of token-sharding. Each core emits a PARTIAL
bf16 output (its heads' contribution through w_o); the host sums the
four partials per batch and adds the V/O bias term (b_v @ w_o.T +
b_o), which is an exact per-token constant. Q/K biases are added
on-device via rank-1 matmuls.

Attention is flash-style with scores kept transposed [tk, tq]. The
two heads' score matmuls (contraction 64) are issued back-to-back at
PE row groups 0/64, so they run CONCURRENTLY in the systolic array
(row tiling) - scores cost half of token-sharding. Softmax skips the
max-subtraction (scores ~ N(0,1)) and the denominator comes from a
ones column appended to V, so softmax is exactly one ACT pass per
score block. The kernel is ACT(exp)-bound: 256 activations of
[128,1024] (sustained ~1.05us each at 1.2 GHz) are the critical
path. Schedule rules discovered on HW: emit next-group scores before
anything else after each exp; never share a PSUM bank between a DVE
read and in-flight PE writes (fatal PSUM collision); keep the
reciprocal on many partitions ([128,8] via a DRAM bounce - a [1,512]
single-lane reciprocal is 3.3us and stalls the DVE queue); warm the
PE through the DMA prefix and the tail bounce so HAM stays at 2.4
GHz.
"""

import numpy as np
import ml_dtypes

B, S, D = 2, 4096, 512
H, DK = 8, 64
N_CORES = 8
PD = 128  # dims per core (2 heads x 64)
NTQ = 8  # tq tiles of 512
NCH = 32  # tk chunks of 128

_PROGRAM = None


def _build_program():
    from contextlib import ExitStack

    import concourse.mybir as mybir
    import concourse.tile as tile
    from concourse import bacc

    bf = mybir.dt.bfloat16
    f32 = mybir.dt.float32
    Exp = mybir.ActivationFunctionType.Exp

    nc = bacc.Bacc(None)

    qT = nc.declare_dram_parameter("qT", [D, S], bf, isOutput=False)
    kT = nc.declare_dram_parameter("kT", [D, S], bf, isOutput=False)
    vT = nc.declare_dram_parameter("vT", [D, S], bf, isOutput=False)
    wqT = nc.declare_dram_parameter("wqT", [D, PD], bf, isOutput=False)
    wkT = nc.declare_dram_parameter("wkT", [D, PD], bf, isOutput=False)
    wvT = nc.declare_dram_parameter("wvT", [D, PD], bf, isOutput=False)
    woT = nc.declare_dram_parameter("woT", [PD, D], bf, isOutput=False)
    bq = nc.declare_dram_parameter("bq", [1, PD], bf, isOutput=False)
    bk = nc.declare_dram_parameter("bk", [1, PD], bf, isOutput=False)
    out_p = nc.declare_dram_parameter("out", [S, D], bf, isOutput=True)
    # DRAM scratch for the denominator-reciprocal partition spread
    rden = nc.dram_tensor("rden", [NTQ, 1024], f32)
    rrec = nc.dram_tensor("rrec", [NTQ, 1024], f32)

    with tile.TileContext(nc) as tc, ExitStack() as ctx:
        wpool = ctx.enter_context(tc.tile_pool(name="w", bufs=1))
        kstream = ctx.enter_context(tc.tile_pool(name="kstream", bufs=2))
        qstream = ctx.enter_context(tc.tile_pool(name="qstream", bufs=2))
        vstream = ctx.enter_context(tc.tile_pool(name="vstream", bufs=2))
        khpool = ctx.enter_context(tc.tile_pool(name="kh", bufs=1))
        qhpool = ctx.enter_context(tc.tile_pool(name="qh", bufs=1))
        vstore = ctx.enter_context(tc.tile_pool(name="vstore", bufs=33))
        ptpool = ctx.enter_context(tc.tile_pool(name="pt", bufs=4))
        opool = ctx.enter_context(tc.tile_pool(name="o", bufs=4))
        wsp = ctx.enter_context(tc.tile_pool(name="ws", bufs=6))
        ostage = ctx.enter_context(tc.tile_pool(name="ostage", bufs=2))
        scorep = ctx.enter_context(tc.tile_pool(name="scorep", bufs=2, space="PSUM"))
        pvp = ctx.enter_context(tc.tile_pool(name="pvp", bufs=2, space="PSUM"))
        projp = ctx.enter_context(tc.tile_pool(name="projp", bufs=2, space="PSUM"))

        dma = nc.sync.dma_start
        MM = nc.tensor.matmul

        # ---- constants (DMA order = sync-queue order = the startup
        # critical path: K-proj inputs first, then Q, then V, the rest) ----
        ones1 = wpool.tile([1, D], bf, tag="ones", name="ones1")
        nc.vector.memset(ones1[:], 1.0)
        # warm the ACT table (exp) during the DMA-heavy prefix
        wrm = wsp.tile([1, 16], f32, tag="denb", name="warm")
        nc.vector.memset(wrm[:], 0.0)
        wrm2 = wsp.tile([1, 16], f32, tag="rrow", name="warm2")
        nc.scalar.activation(out=wrm2[:], in_=wrm[:], func=Exp, scale=1.0)
        # keep the PE busy through the DMA-bound prefix so HAM reaches
        # K=8/8 before (and stays there for) the real matmul stream
        wps = projp.tile([128, 512], f32, tag="proj", name="warm_ps")
        for _ in range(24):
            MM(
                wps[0:1, :],
                ones1[0:1, 0:1],
                ones1[0:1, :],
                start=True,
                stop=True,
                skip_group_check=True,
            )

        def wtiles(param, tagp):
            t = wpool.tile([128, 4, PD], bf, tag=tagp, name=tagp)
            dma(out=t[:], in_=param[:].rearrange("(c p) d -> p c d", p=128))
            return t

        khT = khpool.tile([PD, S], bf, tag="khT", name="khT")
        qhT = qhpool.tile([PD, S], bf, tag="qhT", name="qhT")
        v_store = [None] * NCH  # [128 tok, 2 heads, DK+1]; col 64 = ones

        kraw_t = {}
        qraw_t = {}
        vraw_t = {}

        def dma_kraw(t):
            kr = kstream.tile([128, 4, 512], bf, tag="kraw", name="kraw")
            dma(
                out=kr[:],
                in_=kT[:, t * 512 : (t + 1) * 512].rearrange(
                    "(c p) t -> p c t", p=128
                ),
            )
            kraw_t[t] = kr

        def dma_qraw(t):
            qr = qstream.tile([128, 4, 512], bf, tag="qraw", name="qraw")
            dma(
                out=qr[:],
                in_=qT[:, t * 512 : (t + 1) * 512].rearrange(
                    "(c p) t -> p c t", p=128
                ),
            )
            qraw_t[t] = qr

        def dma_vraw(r):
            vr = vstream.tile([128, 4, 512], bf, tag="vraw", name="vraw")
            dma(
                out=vr[:],
                in_=vT[:, r * 512 : (r + 1) * 512].rearrange(
                    "(c p) t -> p c t", p=128
                ),
            )
            vraw_t[r] = vr

        def proj_qk(raw, w_t, b_t, dst, t):
            """Project K or Q for token tile t -> dst[:, t*512:(t+1)*512]."""
            ps = projp.tile([128, 512], f32, tag="proj", name="proj_ps")
            for kk in range(4):
                MM(
                    ps[:],
                    w_t[:, kk, :],
                    raw[:, kk, :],
                    start=(kk == 0),
                    stop=False,
                    skip_group_check=True,
                )
            MM(
                ps[:],
                b_t[:],
                ones1[0:1, 0:512],
                start=False,
                stop=True,
                skip_group_check=True,
            )
            nc.vector.tensor_copy(out=dst[:, t * 512 : (t + 1) * 512], in_=ps[:])

        def proj_v_sub(r, sub):
            """Project V tokens (4r+sub)*128.. into v_store[4r+sub].

            Fresh PSUM tile per sub-chunk: sharing one bank across
            sub-chunks makes the DVE copy of chunk n concurrent with PE
            writes of chunk n+1 in the same bank (fatal PSUM collision).
            """
            j = 4 * r + sub
            ps = projp.tile([128, 512], f32, tag="proj", name="vps")
            for kk in range(4):
                MM(
                    ps[:, 0:128],
                    vraw_t[r][:, kk, sub * 128 : (sub + 1) * 128],
                    wv_t[:, kk, :],
                    start=(kk == 0),
                    stop=(kk == 3),
                    skip_group_check=True,
                )
            vs = vstore.tile([128, 2, DK + 1], bf, tag="vs", name="vs")
            v_store[j] = vs
            nc.vector.memset(vs[:, :, DK : DK + 1], 1.0)
            nc.vector.tensor_copy(
                out=vs[:, :, 0:DK],
                in_=ps[:, 0:128].rearrange("p (h c) -> p h c", c=DK),
            )

        def emit_scores(tqt, j):
            sc = scorep.tile([128, 1024], f32, tag="sc", name="sc")
            for h in range(2):
                pb = h * 64
                MM(
                    sc[:, h * 512 : (h + 1) * 512],
                    khT[pb : pb + 64, j * 128 : (j + 1) * 128],
                    qhT[pb : pb + 64, tqt * 512 : (tqt + 1) * 512],
                    start=True,
                    stop=True,
                    skip_group_check=True,
                )
            return sc

        # ---- closure schedule: group index -> list of closures ----
        extra = {}

        def add(g, fn):
            extra.setdefault(g, []).append(fn)

        # K tiles 1-7: dma 4 groups ahead of the matmuls
        for t in range(1, 8):
            add(4 * t - 4, lambda t=t: dma_kraw(t))
            add(
                4 * t - 2,
                lambda t=t: proj_qk(kraw_t[t], wk_t, bk_t, khT, t),
            )
        # V raw streams r=2..7 (r=0,1 in prefix); sub-closure for chunk j
        # runs at group j-4
        for r in range(2, 8):
            add(4 * r - 8, lambda r=r: dma_vraw(r))
        for j in range(4, NCH):
            add(j - 4, lambda r=j // 4, s=j % 4: proj_v_sub(r, s))
        # Q tiles 1-7 projected near the end of the previous tq tile
        for T in range(1, 8):
            add((T - 1) * 32 + 24, lambda T=T: dma_qraw(T))
            add(
                (T - 1) * 32 + 26,
                lambda T=T: proj_qk(qraw_t[T], wq_t, bq_t, qhT, T),
            )

        # ---- tile close + normalization/out-projection ----
        # At tile close: copy both PV accumulators to SBUF (frees the
        # PSUM banks for the next tile), pack the two denominator rows
        # into one [2,512] tile and bounce it through DRAM to spread it
        # as [128,8] (h*64+p lanes) so the reciprocal runs 8 elems/lane.
        # One bounce for both heads keeps the sync queue light.
        def emit_close(tqt, pv_tiles):
            pvsb = []
            for h in range(2):
                t = wsp.tile([DK + 1, 512], f32, tag="pvsb", name="pvsb")
                nc.vector.tensor_copy(out=t[:], in_=pv_tiles[h][:])
                pvsb.append(t)
            dsb = wsp.tile([1, 1024], f32, tag="denb", name="denb")
            for h in range(2):
                nc.vector.tensor_copy(
                    out=dsb[0:1, h * 512 : (h + 1) * 512], in_=pvsb[h][64:65, :]
                )
            dma(out=rden[tqt : tqt + 1, :], in_=dsb[:])
            sp = wsp.tile([128, 8], f32, tag="sp", name="sp")
            dma(out=sp[:], in_=rden[tqt].rearrange("(p e) -> p e", p=128))
            sp2 = wsp.tile([128, 8], f32, tag="sp2", name="sp2")
            nc.vector.reciprocal(out=sp2[:], in_=sp[:])
            dma(out=rrec[tqt].rearrange("(p e) -> p e", p=128), in_=sp2[:])
            return pvsb

        def make_close_steps(tqt, pvsb):
            o_n = [None, None]
            steps = []
            for h in range(2):

                def s2(h=h):
                    w = wsp.tile([64, 512], f32, tag="ws", name="wst")
                    dma(
                        out=w[:],
                        in_=rrec[
                            tqt : tqt + 1, h * 512 : (h + 1) * 512
                        ].partition_broadcast(64),
                    )
                    on = opool.tile([DK, 512], bf, tag="oh", name="oh")
                    nc.vector.tensor_mul(out=on[:], in0=pvsb[h][0:64, :], in1=w[:])
                    o_n[h] = on

                steps.append(s2)
            for tt in range(4):

                def s3(tt=tt):
                    pa = projp.tile([128, 512], f32, tag="proj", name="pa")
                    for h in range(2):
                        MM(
                            pa[:],
                            o_n[h][:, tt * 128 : (tt + 1) * 128],
                            wo_t[:, h, :],
                            start=(h == 0),
                            stop=(h == 1),
                            skip_group_check=True,
                        )
                    ot = ostage.tile([128, 512], bf, tag="ot", name="ot")
                    nc.vector.tensor_copy(out=ot[:], in_=pa[:])
                    dma(
                        out=out_p[
                            tqt * 512 + tt * 128 : tqt * 512 + (tt + 1) * 128, :
                        ],
                        in_=ot[:],
                    )

                steps.append(s3)
            return steps

        CLOSE_SLOTS = (3, 4, 7, 11, 15, 19)
        pend = {}

        # ---- prefix: interleave input DMAs with the projections that
        # consume them so the first exp fires as early as possible ----
        dma_kraw(0)
        wk_t = wtiles(wkT, "wk")
        bk_t = wpool.tile([1, PD], bf, tag="bk", name="bk_t")
        dma(out=bk_t[:], in_=bk[:])
        dma_qraw(0)
        wq_t = wtiles(wqT, "wq")
        bq_t = wpool.tile([1, PD], bf, tag="bq", name="bq_t")
        dma(out=bq_t[:], in_=bq[:])
        dma_vraw(0)
        wv_t = wtiles(wvT, "wv")
        # [64, 2, D]: wo_t[:, h, :] puts both heads' w_o rows at base
        # partition 0, so out-proj MMs share row group 0 (concurrent
        # row-group accumulation into one PSUM bank races).
        wo_t = wpool.tile([DK, 2, D], bf, tag="wo", name="wo_t")
        dma(out=wo_t[:], in_=woT[:].rearrange("(h p) d -> p h d", p=DK))
        dma_vraw(1)
        proj_qk(kraw_t[0], wk_t, bk_t, khT, 0)
        proj_qk(qraw_t[0], wq_t, bq_t, qhT, 0)
        for sub in range(4):
            proj_v_sub(0, sub)

        # ---- main attention loop ----
        sc_next = emit_scores(0, 0)
        pv_tiles = None
        for g in range(NTQ * NCH):
            tqt, j = divmod(g, NCH)
            if j == 0:
                pv_tiles = [
                    pvp.tile([DK + 1, 512], f32, tag="pv", name=f"pv{_h}")
                    for _h in range(2)
                ]
            sc = sc_next
            pt = ptpool.tile([128, 1024], bf, tag="pt", name="pt")
            nc.scalar.activation(out=pt[:], in_=sc[:], func=Exp, scale=0.125)
            # next group's scores first: nothing else may delay the PE
            # work that feeds ACT
            if g + 1 < NTQ * NCH:
                ntqt, nj = divmod(g + 1, NCH)
                sc_next = emit_scores(ntqt, nj)
            # interleaved work (projections, previous tile's out-proj)
            for fn in extra.get(g, ()):
                fn()
            if tqt >= 1 and j in CLOSE_SLOTS and (tqt - 1) in pend:
                pend[tqt - 1][CLOSE_SLOTS.index(j)]()
            for h in range(2):
                MM(
                    pv_tiles[h][:],
                    v_store[j][:, h, :],
                    pt[:, h * 512 : (h + 1) * 512],
                    start=(j == 0),
                    stop=(j == NCH - 1),
                    skip_group_check=True,
                )
            if j == NCH - 1:
                o_u = emit_close(tqt, pv_tiles)
                pend[tqt] = make_close_steps(tqt, o_u)

        # ---- tail: last tq tile's normalization + out-projection ----
        # Filler matmuls keep HAM at K=8/8 while the reciprocal bounce
        # round-trips, so the final out-projections run at 2.4 GHz.
        wps2 = projp.tile([128, 512], f32, tag="proj", name="tail_warm")
        for _ in range(10):
            MM(
                wps2[0:1, :],
                ones1[0:1, 0:1],
                ones1[0:1, :],
                start=True,
                stop=True,
                skip_group_check=True,
            )
        for fn in pend[NTQ - 1]:
            fn()

    if not nc.is_finalized():
        nc.finalize()
    return nc


def _get_program():
    global _PROGRAM
    if _PROGRAM is None:
        _PROGRAM = _build_program()
    return _PROGRAM


def _prep_inputs(q, k, v, w_q, b_q, w_k, b_k, w_v, b_v, w_o, b_o):
    bf16 = ml_dtypes.bfloat16
    q = np.asarray(q, dtype=np.float32)
    k = np.asarray(k, dtype=np.float32)
    v = np.asarray(v, dtype=np.float32)
    w_q = np.asarray(w_q, np.float32)
    w_k = np.asarray(w_k, np.float32)
    w_v = np.asarray(w_v, np.float32)
    w_o = np.asarray(w_o, np.float32)
    b_q = np.asarray(b_q, np.float32)
    b_k = np.asarray(b_k, np.float32)
    b_v = np.asarray(b_v, np.float32)
    b_o = np.asarray(b_o, np.float32)

    qT = [np.ascontiguousarray(q[b].T).astype(bf16) for b in range(B)]
    kTb = [np.ascontiguousarray(k[b].T).astype(bf16) for b in range(B)]
    vTb = [np.ascontiguousarray(v[b].T).astype(bf16) for b in range(B)]
    wqT = np.ascontiguousarray(w_q.T).astype(bf16)  # [D_in, D_out]
    wkT = np.ascontiguousarray(w_k.T).astype(bf16)
    wvT = np.ascontiguousarray(w_v.T).astype(bf16)
    woT = np.ascontiguousarray(w_o.T)  # [D_in(head dims), D_out] f32

    in_maps = []
    for c in range(N_CORES):
        b, p = divmod(c, 4)
        ds = slice(p * PD, (p + 1) * PD)
        in_maps.append(
            {
                "qT": qT[b],
                "kT": kTb[b],
                "vT": vTb[b],
                "wqT": np.ascontiguousarray(wqT[:, ds]),
                "wkT": np.ascontiguousarray(wkT[:, ds]),
                "wvT": np.ascontiguousarray(wvT[:, ds]),
                "woT": np.ascontiguousarray(woT[ds, :]).astype(bf16),
                "bq": b_q[ds].reshape(1, PD).astype(bf16),
                "bk": b_k[ds].reshape(1, PD).astype(bf16),
            }
        )
    # V/O biases are exact per-token constants: fold on the host instead
    # of spending device matmuls (partials exclude them; added in combine)
    bias_full = (b_v @ woT + b_o).astype(np.float32)
    return in_maps, bias_full


def run_cores(in_maps, trace=False, **kw):
    """Compile+run the SPMD program; returns BassKernelResults."""
    from concourse.bass_utils import run_bass_kernel_spmd

    nc = _get_program()
    return run_bass_kernel_spmd(nc, in_maps, list(range(N_CORES)), trace=trace, **kw)


def combine_outputs(res, bias_full):
    """Sum the per-core partial outputs into the full [B, S, D] result."""
    out = np.zeros((B, S, D), np.float32)
    for c in range(N_CORES):
        b = c // 4
        out[b] += res.results[c]["out"]
    out += bias_full
    return out


def kernel(q, k, v, w_q, b_q, w_k, b_k, w_v, b_v, w_o, b_o):
    in_maps, bias_full = _prep_inputs(
        q, k, v, w_q, b_q, w_k, b_k, w_v, b_v, w_o, b_o
    )
    res = run_cores(in_maps)
    return combine_outputs(res, bias_full)


# revision 34
# speedup vs baseline: 1.1580x; 1.0263x over previous
"""Multi-head attention (B=2, S=4096, D=512, H=8) on 8 TRN2 NeuronCores.

Sharding: batch x head-pair (tensor parallel). Core c handles batch
b=c//4 and heads {2p, 2p+1} with p=c%4, over the FULL 4096-token
sequence. Q/K/V/O projections are sliced along the head dimension
(each core projects only its 128 dims), eliminating the redundant
full K/V projection of token-sharding. Each core emits a PARTIAL
bf16 output (its heads' contribution through w_o); the host sums the
four partials per batch and adds the V/O bias term (b_v @ w_o.T +
b_o), which is an exact per-token constant. Q/K biases are added
on-device via rank-1 matmuls.

Attention is flash-style with scores kept transposed [tk, tq]. The
two heads' score matmuls (contraction 64) are issued back-to-back at
PE row groups 0/64, so they run CONCURRENTLY in the systolic array
(row tiling) - scores cost half of token-sharding. Softmax skips the
max-subtraction (scores ~ N(0,1)) and the denominator comes from a
ones column appended to V, so softmax is exactly one ACT pass per
score block. The kernel is ACT(exp)-bound: 256 activations of
[128,1024] (sustained ~1.05us each at 1.2 GHz) are the critical
path. Schedule rules discovered on HW: emit next-group scores before
anything else after each exp; never share a PSUM bank between a DVE
read and in-flight PE writes (fatal PSUM collision); keep the
reciprocal on many partitions ([128,8] via a DRAM bounce - a [1,512]
single-lane reciprocal is 3.3us and stalls the DVE queue); warm the
PE through the DMA prefix and the tail bounce so HAM stays at 2.4
GHz.
"""

import numpy as np
import ml_dtypes

B, S, D = 2, 4096, 512
H, DK = 8, 64
N_CORES = 8
PD = 128  # dims per core (2 heads x 64)
NTQ = 8  # tq tiles of 512
NCH = 32  # tk chunks of 128

_PROGRAM = None


def _build_program():
    from contextlib import ExitStack

    import concourse.mybir as mybir
    import concourse.tile as tile
    from concourse import bacc

    bf = mybir.dt.bfloat16
    f32 = mybir.dt.float32
    Exp = mybir.ActivationFunctionType.Exp

    nc = bacc.Bacc(None)

    qT = nc.declare_dram_parameter("qT", [D, S], bf, isOutput=False)
    kT = nc.declare_dram_parameter("kT", [D, S], bf, isOutput=False)
    vT = nc.declare_dram_parameter("vT", [D, S], bf, isOutput=False)
    wqT = nc.declare_dram_parameter("wqT", [D, PD], bf, isOutput=False)
    wkT = nc.declare_dram_parameter("wkT", [D, PD], bf, isOutput=False)
    wvT = nc.declare_dram_parameter("wvT", [D, PD], bf, isOutput=False)
    woT = nc.declare_dram_parameter("woT", [PD, D], bf, isOutput=False)
    bq = nc.declare_dram_parameter("bq", [1, PD], bf, isOutput=False)
    bk = nc.declare_dram_parameter("bk", [1, PD], bf, isOutput=False)
    out_p = nc.declare_dram_parameter("out", [S, D], bf, isOutput=True)
    # DRAM scratch for the denominator-reciprocal partition spread
    rden = nc.dram_tensor("rden", [NTQ, 1024], f32)
    rrec = nc.dram_tensor("rrec", [NTQ, 1024], f32)

    with tile.TileContext(nc) as tc, ExitStack() as ctx:
        wpool = ctx.enter_context(tc.tile_pool(name="w", bufs=1))
        kstream = ctx.enter_context(tc.tile_pool(name="kstream", bufs=2))
        qstream = ctx.enter_context(tc.tile_pool(name="qstream", bufs=2))
        vstream = ctx.enter_context(tc.tile_pool(name="vstream", bufs=2))
        khpool = ctx.enter_context(tc.tile_pool(name="kh", bufs=1))
        qhpool = ctx.enter_context(tc.tile_pool(name="qh", bufs=1))
        vstore = ctx.enter_context(tc.tile_pool(name="vstore", bufs=33))
        ptpool = ctx.enter_context(tc.tile_pool(name="pt", bufs=4))
        opool = ctx.enter_context(tc.tile_pool(name="o", bufs=4))
        wsp = ctx.enter_context(tc.tile_pool(name="ws", bufs=6))
        ostage = ctx.enter_context(tc.tile_pool(name="ostage", bufs=2))
        scorep = ctx.enter_context(tc.tile_pool(name="scorep", bufs=2, space="PSUM"))
        pvp = ctx.enter_context(tc.tile_pool(name="pvp", bufs=2, space="PSUM"))
        projp = ctx.enter_context(tc.tile_pool(name="projp", bufs=2, space="PSUM"))

        dma = nc.sync.dma_start
        MM = nc.tensor.matmul

        # ---- constants (DMA order = sync-queue order = the startup
        # critical path: K-proj inputs first, then Q, then V, the rest) ----
        ones1 = wpool.tile([1, D], bf, tag="ones", name="ones1")
        nc.vector.memset(ones1[:], 1.0)
        # warm the ACT table (exp) during the DMA-heavy prefix
        wrm = wsp.tile([1, 16], f32, tag="denb", name="warm")
        nc.vector.memset(wrm[:], 0.0)
        wrm2 = wsp.tile([1, 16], f32, tag="rrow", name="warm2")
        nc.scalar.activation(out=wrm2[:], in_=wrm[:], func=Exp, scale=1.0)
        # keep the PE busy through the DMA-bound prefix so HAM reaches
        # K=8/8 before (and stays there for) the real matmul stream
        zeros_t = wpool.tile([128, 512], f32, tag="zeros", name="zeros_t")
        nc.vector.memset(zeros_t[:], 0.0)
        wps = projp.tile([128, 512], f32, tag="proj", name="warm_ps")
        for _ in range(10):
            MM(
                wps[0:1, :],
                ones1[0:1, 0:1],
                ones1[0:1, :],
                start=True,
                stop=True,
                skip_group_check=True,
            )

        def wtiles(param, tagp):
            t = wpool.tile([128, 4, PD], bf, tag=tagp, name=tagp)
            dma(out=t[:], in_=param[:].rearrange("(c p) d -> p c d", p=128))
            return t

        khT = khpool.tile([PD, S], bf, tag="khT", name="khT")
        qhT = qhpool.tile([PD, S], bf, tag="qhT", name="qhT")
        v_store = [None] * NCH  # [128 tok, 2 heads, DK+1]; col 64 = ones

        kraw_t = {}
        qraw_t = {}
        vraw_t = {}

        def dma_kraw(t):
            kr = kstream.tile([128, 4, 512], bf, tag="kraw", name="kraw")
            dma(
                out=kr[:],
                in_=kT[:, t * 512 : (t + 1) * 512].rearrange(
                    "(c p) t -> p c t", p=128
                ),
            )
            kraw_t[t] = kr

        def dma_qraw(t):
            qr = qstream.tile([128, 4, 512], bf, tag="qraw", name="qraw")
            dma(
                out=qr[:],
                in_=qT[:, t * 512 : (t + 1) * 512].rearrange(
                    "(c p) t -> p c t", p=128
                ),
            )
            qraw_t[t] = qr

        def dma_vraw(r):
            vr = vstream.tile([128, 4, 512], bf, tag="vraw", name="vraw")
            dma(
                out=vr[:],
                in_=vT[:, r * 512 : (r + 1) * 512].rearrange(
                    "(c p) t -> p c t", p=128
                ),
            )
            vraw_t[r] = vr

        def proj_qk(raw, w_t, b_t, dst, t):
            """Project K or Q for token tile t -> dst[:, t*512:(t+1)*512]."""
            ps = projp.tile([128, 512], f32, tag="proj", name="proj_ps")
            for kk in range(4):
                MM(
                    ps[:],
                    w_t[:, kk, :],
                    raw[:, kk, :],
                    start=(kk == 0),
                    stop=False,
                    skip_group_check=True,
                )
            MM(
                ps[:],
                b_t[:],
                ones1[0:1, 0:512],
                start=False,
                stop=True,
                skip_group_check=True,
            )
            nc.vector.tensor_copy(out=dst[:, t * 512 : (t + 1) * 512], in_=ps[:])

        def proj_v_sub(r, sub):
            """Project V tokens (4r+sub)*128.. into v_store[4r+sub].

            Fresh PSUM tile per sub-chunk: sharing one bank across
            sub-chunks makes the DVE copy of chunk n concurrent with PE
            writes of chunk n+1 in the same bank (fatal PSUM collision).
            """
            j = 4 * r + sub
            ps = projp.tile([128, 512], f32, tag="proj", name="vps")
            for kk in range(4):
                MM(
                    ps[:, 0:128],
                    vraw_t[r][:, kk, sub * 128 : (sub + 1) * 128],
                    wv_t[:, kk, :],
                    start=(kk == 0),
                    stop=(kk == 3),
                    skip_group_check=True,
                )
            vs = vstore.tile([128, 2, DK + 1], bf, tag="vs", name="vs")
            v_store[j] = vs
            nc.vector.memset(vs[:, :, DK : DK + 1], 1.0)
            nc.vector.tensor_copy(
                out=vs[:, :, 0:DK],
                in_=ps[:, 0:128].rearrange("p (h c) -> p h c", c=DK),
            )

        def emit_scores(tqt, j):
            sc = scorep.tile([128, 1024], f32, tag="sc", name="sc")
            for h in range(2):
                pb = h * 64
                MM(
                    sc[:, h * 512 : (h + 1) * 512],
                    khT[pb : pb + 64, j * 128 : (j + 1) * 128],
                    qhT[pb : pb + 64, tqt * 512 : (tqt + 1) * 512],
                    start=True,
                    stop=True,
                    skip_group_check=True,
                )
            return sc

        # ---- closure schedule: group index -> list of closures ----
        extra = {}

        def add(g, fn):
            extra.setdefault(g, []).append(fn)

        # K tiles 1-7: dma 4 groups ahead of the matmuls
        for t in range(1, 8):
            add(4 * t - 4, lambda t=t: dma_kraw(t))
            add(
                4 * t - 2,
                lambda t=t: proj_qk(kraw_t[t], wk_t, bk_t, khT, t),
            )
        # V raw streams r=2..7 (r=0,1 in prefix); chunk j's projection at
        # group j-1 (one per group - its MMs precede PV(j-1), and PV(j)
        # only runs a full group later)
        for r in range(2, 8):
            add(4 * r - 6, lambda r=r: dma_vraw(r))
        for j in range(1, NCH):
            add(j - 1, lambda r=j // 4, s=j % 4: proj_v_sub(r, s))
        # Q tiles 1-7 projected near the end of the previous tq tile
        for T in range(1, 8):
            add((T - 1) * 32 + 24, lambda T=T: dma_qraw(T))
            add(
                (T - 1) * 32 + 26,
                lambda T=T: proj_qk(qraw_t[T], wq_t, bq_t, qhT, T),
            )

        # ---- tile close + normalization/out-projection ----
        # At tile close: copy both PV accumulators to SBUF (frees the
        # PSUM banks for the next tile), pack the two denominator rows
        # into one [2,512] tile and bounce it through DRAM to spread it
        # as [128,8] (h*64+p lanes) so the reciprocal runs 8 elems/lane.
        # One bounce for both heads keeps the sync queue light.
        def emit_close(tqt, pv_tiles):
            pvsb = []
            for h in range(2):
                t = wsp.tile([DK + 1, 512], f32, tag="pvsb", name="pvsb")
                nc.vector.tensor_copy(out=t[:], in_=pv_tiles[h][:])
                pvsb.append(t)
            dsb = wsp.tile([1, 1024], f32, tag="denb", name="denb")
            for h in range(2):
                nc.vector.tensor_copy(
                    out=dsb[0:1, h * 512 : (h + 1) * 512], in_=pvsb[h][64:65, :]
                )
            dma(out=rden[tqt : tqt + 1, :], in_=dsb[:])
            sp = wsp.tile([128, 8], f32, tag="sp", name="sp")
            dma(out=sp[:], in_=rden[tqt].rearrange("(p e) -> p e", p=128))
            sp2 = wsp.tile([128, 8], f32, tag="sp2", name="sp2")
            nc.vector.reciprocal(out=sp2[:], in_=sp[:])
            dma(out=rrec[tqt].rearrange("(p e) -> p e", p=128), in_=sp2[:])
            return pvsb

        def make_close_steps(tqt, pvsb):
            o_n = [None, None]
            steps = []
            for h in range(2):

                def s2(h=h):
                    w = wsp.tile([64, 512], f32, tag="ws", name="wst")
                    dma(
                        out=w[:],
                        in_=rrec[
                            tqt : tqt + 1, h * 512 : (h + 1) * 512
                        ].partition_broadcast(64),
                    )
                    on = opool.tile([DK, 512], bf, tag="oh", name="oh")
                    nc.vector.tensor_mul(out=on[:], in0=pvsb[h][0:64, :], in1=w[:])
                    o_n[h] = on

                steps.append(s2)
            for tt in range(4):

                def s3(tt=tt):
                    pa = projp.tile([128, 512], f32, tag="proj", name="pa")
                    for h in range(2):
                        MM(
                            pa[:],
                            o_n[h][:, tt * 128 : (tt + 1) * 128],
                            wo_t[:, h, :],
                            start=(h == 0),
                            stop=(h == 1),
                            skip_group_check=True,
                        )
                    ot = ostage.tile([128, 512], bf, tag="ot", name="ot")
                    nc.vector.tensor_copy(out=ot[:], in_=pa[:])
                    dma(
                        out=out_p[
                            tqt * 512 + tt * 128 : tqt * 512 + (tt + 1) * 128, :
                        ],
                        in_=ot[:],
                    )

                steps.append(s3)
            return steps

        CLOSE_SLOTS = (3, 4, 7, 11, 15, 19)
        pend = {}

        # ---- prefix: interleave input DMAs with the projections that
        # consume them so the first exp fires as early as possible ----
        dma_kraw(0)
        wk_t = wtiles(wkT, "wk")
        bk_t = wpool.tile([1, PD], bf, tag="bk", name="bk_t")
        dma(out=bk_t[:], in_=bk[:])
        dma_qraw(0)
        wq_t = wtiles(wqT, "wq")
        bq_t = wpool.tile([1, PD], bf, tag="bq", name="bq_t")
        dma(out=bq_t[:], in_=bq[:])
        dma_vraw(0)
        wv_t = wtiles(wvT, "wv")
        # [64, 2, D]: wo_t[:, h, :] puts both heads' w_o rows at base
        # partition 0, so out-proj MMs share row group 0 (concurrent
        # row-group accumulation into one PSUM bank races).
        wo_t = wpool.tile([DK, 2, D], bf, tag="wo", name="wo_t")
        dma(out=wo_t[:], in_=woT[:].rearrange("(h p) d -> p h d", p=DK))
        dma_vraw(1)
        proj_qk(kraw_t[0], wk_t, bk_t, khT, 0)
        proj_qk(qraw_t[0], wq_t, bq_t, qhT, 0)

        # ---- main attention loop ----
        sc_next = emit_scores(0, 0)
        # V chunk 0 projected after the first scores: exp(0) must not
        # queue behind V matmuls, and PV(0,0) only needs it post-exp
        proj_v_sub(0, 0)
        pv_tiles = None
        for g in range(NTQ * NCH):
            tqt, j = divmod(g, NCH)
            if j == 0:
                pv_tiles = [
                    pvp.tile([DK + 1, 512], f32, tag="pv", name=f"pv{_h}")
                    for _h in range(2)
                ]
            sc = sc_next
            pt = ptpool.tile([128, 1024], bf, tag="pt", name="pt")
            nc.scalar.activation(out=pt[:], in_=sc[:], func=Exp, scale=0.125)
            # next group's scores first: nothing else may delay the PE
            # work that feeds ACT
            if g + 1 < NTQ * NCH:
                ntqt, nj = divmod(g + 1, NCH)
                sc_next = emit_scores(ntqt, nj)
            # interleaved work (projections, previous tile's out-proj)
            for fn in extra.get(g, ()):
                fn()
            if tqt >= 1 and j in CLOSE_SLOTS and (tqt - 1) in pend:
                pend[tqt - 1][CLOSE_SLOTS.index(j)]()
            for h in range(2):
                MM(
                    pv_tiles[h][:],
                    v_store[j][:, h, :],
                    pt[:, h * 512 : (h + 1) * 512],
                    start=(j == 0),
                    stop=(j == NCH - 1),
                    skip_group_check=True,
                )
            if j == NCH - 1 and tqt < NTQ - 1:
                pvsb = emit_close(tqt, pv_tiles)
                pend[tqt] = make_close_steps(tqt, pvsb)

        # ---- tail: last tile takes a latency-optimized path. The
        # out-projection runs on UNNORMALIZED O right away; the
        # denominators take ONE bounce to spread per-token across
        # partitions, are reciprocal'd there ([128,8], 8 recips worth in
        # one op), and the division is applied afterwards per token row:
        # out = pa*rc0 + pb*rc1 (scalar_tensor_tensor). Filler matmuls
        # keep HAM at K=8/8 through the bounce wait. ----
        tqt = NTQ - 1
        dsb = wsp.tile([1, 1024], f32, tag="denb", name="denb7")
        for h in range(2):
            nc.vector.tensor_copy(
                out=dsb[0:1, h * 512 : (h + 1) * 512], in_=pv_tiles[h][64:65, :]
            )
        o_u7 = []
        for h in range(2):
            ou = opool.tile([DK, 512], bf, tag="oh", name="oh7")
            nc.vector.tensor_copy(out=ou[:], in_=pv_tiles[h][0:64, :])
            o_u7.append(ou)
        dma(out=rden[tqt : tqt + 1, :], in_=dsb[:])
        rc_all = wsp.tile([128, 8], f32, tag="sp", name="rc_all")
        for h in range(2):
            for tt in range(4):
                dma(
                    out=rc_all[:, 4 * h + tt : 4 * h + tt + 1],
                    in_=rden[
                        tqt, h * 512 + tt * 128 : h * 512 + (tt + 1) * 128
                    ].rearrange("(p o) -> p o", o=1),
                )
        rcr = wsp.tile([128, 8], f32, tag="sp2", name="rcr")
        nc.vector.reciprocal(out=rcr[:], in_=rc_all[:])
        wps2 = projp.tile([128, 512], f32, tag="proj", name="tail_warm")
        for _ in range(8):
            MM(
                wps2[0:1, :],
                ones1[0:1, 0:1],
                ones1[0:1, :],
                start=True,
                stop=True,
                skip_group_check=True,
            )
        Mult = mybir.AluOpType.mult
        Add = mybir.AluOpType.add
        for tt in range(4):
            pa = projp.tile([128, 512], f32, tag="proj", name="pa7")
            MM(
                pa[:],
                o_u7[0][:, tt * 128 : (tt + 1) * 128],
                wo_t[:, 0, :],
                start=True,
                stop=True,
                skip_group_check=True,
            )
            pb = projp.tile([128, 512], f32, tag="proj", name="pb7")
            MM(
                pb[:],
                o_u7[1][:, tt * 128 : (tt + 1) * 128],
                wo_t[:, 1, :],
                start=True,
                stop=True,
                skip_group_check=True,
            )
            t1 = ostage.tile([128, 512], f32, tag="ot", name="ot1")
            nc.vector.scalar_tensor_tensor(
                out=t1[:],
                in0=pb[:],
                scalar=rcr[:, 4 + tt : 5 + tt],
                in1=zeros_t[:],
                op0=Mult,
                op1=Add,
            )
            t2 = ostage.tile([128, 512], bf, tag="ot2", name="ot2")
            nc.vector.scalar_tensor_tensor(
                out=t2[:],
                in0=pa[:],
                scalar=rcr[:, tt : tt + 1],
                in1=t1[:],
                op0=Mult,
                op1=Add,
            )
            dma(
                out=out_p[tqt * 512 + tt * 128 : tqt * 512 + (tt + 1) * 128, :],
                in_=t2[:],
            )

    if not nc.is_finalized():
        nc.finalize()
    return nc


def _get_program():
    global _PROGRAM
    if _PROGRAM is None:
        _PROGRAM = _build_program()
    return _PROGRAM


def _prep_inputs(q, k, v, w_q, b_q, w_k, b_k, w_v, b_v, w_o, b_o):
    bf16 = ml_dtypes.bfloat16
    q = np.asarray(q, dtype=np.float32)
    k = np.asarray(k, dtype=np.float32)
    v = np.asarray(v, dtype=np.float32)
    w_q = np.asarray(w_q, np.float32)
    w_k = np.asarray(w_k, np.float32)
    w_v = np.asarray(w_v, np.float32)
    w_o = np.asarray(w_o, np.float32)
    b_q = np.asarray(b_q, np.float32)
    b_k = np.asarray(b_k, np.float32)
    b_v = np.asarray(b_v, np.float32)
    b_o = np.asarray(b_o, np.float32)

    qT = [np.ascontiguousarray(q[b].T).astype(bf16) for b in range(B)]
    kTb = [np.ascontiguousarray(k[b].T).astype(bf16) for b in range(B)]
    vTb = [np.ascontiguousarray(v[b].T).astype(bf16) for b in range(B)]
    wqT = np.ascontiguousarray(w_q.T).astype(bf16)  # [D_in, D_out]
    wkT = np.ascontiguousarray(w_k.T).astype(bf16)
    wvT = np.ascontiguousarray(w_v.T).astype(bf16)
    woT = np.ascontiguousarray(w_o.T)  # [D_in(head dims), D_out] f32

    in_maps = []
    for c in range(N_CORES):
        b, p = divmod(c, 4)
        ds = slice(p * PD, (p + 1) * PD)
        in_maps.append(
            {
                "qT": qT[b],
                "kT": kTb[b],
                "vT": vTb[b],
                "wqT": np.ascontiguousarray(wqT[:, ds]),
                "wkT": np.ascontiguousarray(wkT[:, ds]),
                "wvT": np.ascontiguousarray(wvT[:, ds]),
                "woT": np.ascontiguousarray(woT[ds, :]).astype(bf16),
                "bq": b_q[ds].reshape(1, PD).astype(bf16),
                "bk": b_k[ds].reshape(1, PD).astype(bf16),
            }
        )
    # V/O biases are exact per-token constants: fold on the host instead
    # of spending device matmuls (partials exclude them; added in combine)
    bias_full = (b_v @ woT + b_o).astype(np.float32)
    return in_maps, bias_full


def run_cores(in_maps, trace=False, **kw):
    """Compile+run the SPMD program; returns BassKernelResults."""
    from concourse.bass_utils import run_bass_kernel_spmd

    nc = _get_program()
    return run_bass_kernel_spmd(nc, in_maps, list(range(N_CORES)), trace=trace, **kw)


def combine_outputs(res, bias_full):
    """Sum the per-core partial outputs into the full [B, S, D] result."""
    out = np.zeros((B, S, D), np.float32)
    for c in range(N_CORES):
        b = c // 4
        out[b] += res.results[c]["out"]
    out += bias_full
    return out


def kernel(q, k, v, w_q, b_q, w_k, b_k, w_v, b_v, w_o, b_o):
    in_maps, bias_full = _prep_inputs(
        q, k, v, w_q, b_q, w_k, b_k, w_v, b_v, w_o, b_o
    )
    res = run_cores(in_maps)
    return combine_outputs(res, bias_full)


# revision 35
# speedup vs baseline: 1.1741x; 1.0139x over previous
"""Multi-head attention (B=2, S=4096, D=512, H=8) on 8 TRN2 NeuronCores.

Sharding: batch x head-pair (tensor parallel). Core c handles batch
b=c//4 and heads {2p, 2p+1} with p=c%4, over the FULL 4096-token
sequence. Q/K/V/O projections are sliced along the head dimension
(each core projects only its 128 dims), eliminating the redundant
full K/V projection of token-sharding. Each core emits a PARTIAL
bf16 output (its heads' contribution through w_o); the host sums the
four partials per batch and adds the V/O bias term (b_v @ w_o.T +
b_o), which is an exact per-token constant. Q/K biases are added
on-device via rank-1 matmuls.

Attention is flash-style with scores kept transposed [tk, tq]. The
two heads' score matmuls (contraction 64) are issued back-to-back at
PE row groups 0/64, so they run CONCURRENTLY in the systolic array
(row tiling) - scores cost half of token-sharding. Softmax skips the
max-subtraction (scores ~ N(0,1)) and the denominator comes from a
ones column appended to V, so softmax is exactly one ACT pass per
score block. The kernel is ACT(exp)-bound: 256 activations of
[128,1024] (sustained ~1.05us each at 1.2 GHz) are the critical
path. Schedule rules discovered on HW: emit next-group scores before
anything else after each exp; never share a PSUM bank between a DVE
read and in-flight PE writes (fatal PSUM collision); keep the
reciprocal on many partitions ([128,8] via a DRAM bounce - a [1,512]
single-lane reciprocal is 3.3us and stalls the DVE queue); warm the
PE through the DMA prefix and the tail bounce so HAM stays at 2.4
GHz.
"""

import numpy as np
import ml_dtypes

B, S, D = 2, 4096, 512
H, DK = 8, 64
N_CORES = 8
PD = 128  # dims per core (2 heads x 64)
NTQ = 8  # tq tiles of 512
NCH = 32  # tk chunks of 128

_PROGRAM = None


def _build_program():
    from contextlib import ExitStack

    import concourse.mybir as mybir
    import concourse.tile as tile
    from concourse import bacc

    bf = mybir.dt.bfloat16
    f32 = mybir.dt.float32
    Exp = mybir.ActivationFunctionType.Exp

    nc = bacc.Bacc(None)

    qT = nc.declare_dram_parameter("qT", [D, S], bf, isOutput=False)
    kT = nc.declare_dram_parameter("kT", [D, S], bf, isOutput=False)
    vT = nc.declare_dram_parameter("vT", [D, S], bf, isOutput=False)
    wqT = nc.declare_dram_parameter("wqT", [D, PD], bf, isOutput=False)
    wkT = nc.declare_dram_parameter("wkT", [D, PD], bf, isOutput=False)
    wvT = nc.declare_dram_parameter("wvT", [D, PD], bf, isOutput=False)
    woT = nc.declare_dram_parameter("woT", [PD, D], bf, isOutput=False)
    bq = nc.declare_dram_parameter("bq", [1, PD], bf, isOutput=False)
    bk = nc.declare_dram_parameter("bk", [1, PD], bf, isOutput=False)
    out_p = nc.declare_dram_parameter("out", [S, D], bf, isOutput=True)
    # DRAM scratch for the denominator-reciprocal partition spread
    rden = nc.dram_tensor("rden", [NTQ, 1024], f32)
    rrec = nc.dram_tensor("rrec", [NTQ, 1024], f32)

    with tile.TileContext(nc) as tc, ExitStack() as ctx:
        wpool = ctx.enter_context(tc.tile_pool(name="w", bufs=1))
        kstream = ctx.enter_context(tc.tile_pool(name="kstream", bufs=2))
        qstream = ctx.enter_context(tc.tile_pool(name="qstream", bufs=2))
        vstream = ctx.enter_context(tc.tile_pool(name="vstream", bufs=2))
        khpool = ctx.enter_context(tc.tile_pool(name="kh", bufs=1))
        qhpool = ctx.enter_context(tc.tile_pool(name="qh", bufs=1))
        vstore = ctx.enter_context(tc.tile_pool(name="vstore", bufs=33))
        ptpool = ctx.enter_context(tc.tile_pool(name="pt", bufs=4))
        opool = ctx.enter_context(tc.tile_pool(name="o", bufs=4))
        wsp = ctx.enter_context(tc.tile_pool(name="ws", bufs=6))
        ostage = ctx.enter_context(tc.tile_pool(name="ostage", bufs=2))
        scorep = ctx.enter_context(tc.tile_pool(name="scorep", bufs=2, space="PSUM"))
        pvp = ctx.enter_context(tc.tile_pool(name="pvp", bufs=2, space="PSUM"))
        projp = ctx.enter_context(tc.tile_pool(name="projp", bufs=2, space="PSUM"))

        dma = nc.sync.dma_start
        MM = nc.tensor.matmul

        # ---- constants (DMA order = sync-queue order = the startup
        # critical path: K-proj inputs first, then Q, then V, the rest) ----
        ones1 = wpool.tile([1, D], bf, tag="ones", name="ones1")
        nc.vector.memset(ones1[:], 1.0)
        # warm the ACT table (exp) during the DMA-heavy prefix
        wrm = wsp.tile([1, 16], f32, tag="denb", name="warm")
        nc.vector.memset(wrm[:], 0.0)
        wrm2 = wsp.tile([1, 16], f32, tag="rrow", name="warm2")
        nc.scalar.activation(out=wrm2[:], in_=wrm[:], func=Exp, scale=1.0)
        # keep the PE busy through the DMA-bound prefix so HAM reaches
        # K=8/8 before (and stays there for) the real matmul stream
        zeros_t = wpool.tile([128, 512], f32, tag="zeros", name="zeros_t")
        nc.vector.memset(zeros_t[:], 0.0)
        wps = projp.tile([128, 512], f32, tag="proj", name="warm_ps")
        for _ in range(10):
            MM(
                wps[0:1, :],
                ones1[0:1, 0:1],
                ones1[0:1, :],
                start=True,
                stop=True,
                skip_group_check=True,
            )

        def wtiles(param, tagp):
            t = wpool.tile([128, 4, PD], bf, tag=tagp, name=tagp)
            dma(out=t[:], in_=param[:].rearrange("(c p) d -> p c d", p=128))
            return t

        khT = khpool.tile([PD, S], bf, tag="khT", name="khT")
        qhT = qhpool.tile([PD, S], bf, tag="qhT", name="qhT")
        v_store = [None] * NCH  # [128 tok, 2 heads, DK+1]; col 64 = ones

        kraw_t = {}
        qraw_t = {}
        vraw_t = {}

        def dma_kraw(t):
            kr = kstream.tile([128, 4, 512], bf, tag="kraw", name="kraw")
            dma(
                out=kr[:],
                in_=kT[:, t * 512 : (t + 1) * 512].rearrange(
                    "(c p) t -> p c t", p=128
                ),
            )
            kraw_t[t] = kr

        def dma_qraw(t):
            qr = qstream.tile([128, 4, 512], bf, tag="qraw", name="qraw")
            dma(
                out=qr[:],
                in_=qT[:, t * 512 : (t + 1) * 512].rearrange(
                    "(c p) t -> p c t", p=128
                ),
            )
            qraw_t[t] = qr

        def dma_vraw(r):
            vr = vstream.tile([128, 4, 512], bf, tag="vraw", name="vraw")
            dma(
                out=vr[:],
                in_=vT[:, r * 512 : (r + 1) * 512].rearrange(
                    "(c p) t -> p c t", p=128
                ),
            )
            vraw_t[r] = vr

        def proj_qk(raw, w_t, b_t, dst, t):
            """Project K or Q for token tile t -> dst[:, t*512:(t+1)*512]."""
            ps = projp.tile([128, 512], f32, tag="proj", name="proj_ps")
            for kk in range(4):
                MM(
                    ps[:],
                    w_t[:, kk, :],
                    raw[:, kk, :],
                    start=(kk == 0),
                    stop=False,
                    skip_group_check=True,
                )
            MM(
                ps[:],
                b_t[:],
                ones1[0:1, 0:512],
                start=False,
                stop=True,
                skip_group_check=True,
            )
            nc.vector.tensor_copy(out=dst[:, t * 512 : (t + 1) * 512], in_=ps[:])

        def proj_v_sub(r, sub):
            """Project V tokens (4r+sub)*128.. into v_store[4r+sub].

            Fresh PSUM tile per sub-chunk: sharing one bank across
            sub-chunks makes the DVE copy of chunk n concurrent with PE
            writes of chunk n+1 in the same bank (fatal PSUM collision).
            """
            j = 4 * r + sub
            ps = projp.tile([128, 512], f32, tag="proj", name="vps")
            for kk in range(4):
                MM(
                    ps[:, 0:128],
                    vraw_t[r][:, kk, sub * 128 : (sub + 1) * 128],
                    wv_t[:, kk, :],
                    start=(kk == 0),
                    stop=(kk == 3),
                    skip_group_check=True,
                )
            vs = vstore.tile([128, 2, DK + 1], bf, tag="vs", name="vs")
            v_store[j] = vs
            nc.vector.memset(vs[:, :, DK : DK + 1], 1.0)
            nc.vector.tensor_copy(
                out=vs[:, :, 0:DK],
                in_=ps[:, 0:128].rearrange("p (h c) -> p h c", c=DK),
            )

        def emit_scores(tqt, j):
            sc = scorep.tile([128, 1024], f32, tag="sc", name="sc")
            for h in range(2):
                pb = h * 64
                MM(
                    sc[:, h * 512 : (h + 1) * 512],
                    khT[pb : pb + 64, j * 128 : (j + 1) * 128],
                    qhT[pb : pb + 64, tqt * 512 : (tqt + 1) * 512],
                    start=True,
                    stop=True,
                    skip_group_check=True,
                )
            return sc

        # ---- closure schedule: group index -> list of closures ----
        extra = {}

        def add(g, fn):
            extra.setdefault(g, []).append(fn)

        # K tiles 1-7: dma 4 groups ahead of the matmuls
        for t in range(1, 8):
            add(4 * t - 4, lambda t=t: dma_kraw(t))
            add(
                4 * t - 2,
                lambda t=t: proj_qk(kraw_t[t], wk_t, bk_t, khT, t),
            )
        # V raw streams r=2..7 (r=0,1 in prefix); chunk j's projection at
        # group j-1 (one per group - its MMs precede PV(j-1), and PV(j)
        # only runs a full group later)
        for r in range(2, 8):
            add(4 * r - 6, lambda r=r: dma_vraw(r))
        for j in range(1, NCH):
            add(j - 1, lambda r=j // 4, s=j % 4: proj_v_sub(r, s))
        # Q tiles 1-7 projected near the end of the previous tq tile
        for T in range(1, 8):
            add((T - 1) * 32 + 24, lambda T=T: dma_qraw(T))
            add(
                (T - 1) * 32 + 26,
                lambda T=T: proj_qk(qraw_t[T], wq_t, bq_t, qhT, T),
            )

        # ---- tile close + normalization/out-projection ----
        # At tile close: copy both PV accumulators to SBUF (frees the
        # PSUM banks for the next tile), pack the two denominator rows
        # into one [2,512] tile and bounce it through DRAM to spread it
        # as [128,8] (h*64+p lanes) so the reciprocal runs 8 elems/lane.
        # One bounce for both heads keeps the sync queue light.
        def emit_close(tqt, pv_tiles):
            pvsb = []
            for h in range(2):
                t = wsp.tile([DK + 1, 512], f32, tag="pvsb", name="pvsb")
                nc.vector.tensor_copy(out=t[:], in_=pv_tiles[h][:])
                pvsb.append(t)
            dsb = wsp.tile([1, 1024], f32, tag="denb", name="denb")
            for h in range(2):
                nc.vector.tensor_copy(
                    out=dsb[0:1, h * 512 : (h + 1) * 512], in_=pvsb[h][64:65, :]
                )
            dma(out=rden[tqt : tqt + 1, :], in_=dsb[:])
            sp = wsp.tile([128, 8], f32, tag="sp", name="sp")
            dma(out=sp[:], in_=rden[tqt].rearrange("(p e) -> p e", p=128))
            sp2 = wsp.tile([128, 8], f32, tag="sp2", name="sp2")
            nc.vector.reciprocal(out=sp2[:], in_=sp[:])
            dma(out=rrec[tqt].rearrange("(p e) -> p e", p=128), in_=sp2[:])
            return pvsb

        def make_close_steps(tqt, pvsb):
            o_n = [None, None]
            steps = []
            for h in range(2):

                def s2(h=h):
                    w = wsp.tile([64, 512], f32, tag="ws", name="wst")
                    dma(
                        out=w[:],
                        in_=rrec[
                            tqt : tqt + 1, h * 512 : (h + 1) * 512
                        ].partition_broadcast(64),
                    )
                    on = opool.tile([DK, 512], bf, tag="oh", name="oh")
                    nc.vector.tensor_mul(out=on[:], in0=pvsb[h][0:64, :], in1=w[:])
                    o_n[h] = on

                steps.append(s2)
            for tt in range(4):

                def s3(tt=tt):
                    pa = projp.tile([128, 512], f32, tag="proj", name="pa")
                    for h in range(2):
                        MM(
                            pa[:],
                            o_n[h][:, tt * 128 : (tt + 1) * 128],
                            wo_t[:, h, :],
                            start=(h == 0),
                            stop=(h == 1),
                            skip_group_check=True,
                        )
                    ot = ostage.tile([128, 512], bf, tag="ot", name="ot")
                    nc.vector.tensor_copy(out=ot[:], in_=pa[:])
                    dma(
                        out=out_p[
                            tqt * 512 + tt * 128 : tqt * 512 + (tt + 1) * 128, :
                        ],
                        in_=ot[:],
                    )

                steps.append(s3)
            return steps

        CLOSE_SLOTS = (3, 4, 7, 11, 15, 19)
        pend = {}

        # ---- prefix: interleave input DMAs with the projections that
        # consume them so the first exp fires as early as possible ----
        dma_kraw(0)
        wk_t = wtiles(wkT, "wk")
        bk_t = wpool.tile([1, PD], bf, tag="bk", name="bk_t")
        dma(out=bk_t[:], in_=bk[:])
        dma_qraw(0)
        wq_t = wtiles(wqT, "wq")
        bq_t = wpool.tile([1, PD], bf, tag="bq", name="bq_t")
        dma(out=bq_t[:], in_=bq[:])
        dma_vraw(0)
        wv_t = wtiles(wvT, "wv")
        # [64, 2, D]: wo_t[:, h, :] puts both heads' w_o rows at base
        # partition 0, so out-proj MMs share row group 0 (concurrent
        # row-group accumulation into one PSUM bank races).
        wo_t = wpool.tile([DK, 2, D], bf, tag="wo", name="wo_t")
        dma(out=wo_t[:], in_=woT[:].rearrange("(h p) d -> p h d", p=DK))
        dma_vraw(1)
        proj_qk(kraw_t[0], wk_t, bk_t, khT, 0)
        proj_qk(qraw_t[0], wq_t, bq_t, qhT, 0)

        # ---- main attention loop ----
        sc_next = emit_scores(0, 0)
        # V chunk 0 projected after the first scores: exp(0) must not
        # queue behind V matmuls, and PV(0,0) only needs it post-exp
        proj_v_sub(0, 0)
        pv_tiles = None
        for g in range(NTQ * NCH):
            tqt, j = divmod(g, NCH)
            if j == 0:
                pv_tiles = [
                    pvp.tile([DK + 1, 512], f32, tag="pv", name=f"pv{_h}")
                    for _h in range(2)
                ]
            sc = sc_next
            pt = ptpool.tile([128, 1024], bf, tag="pt", name="pt")
            nc.scalar.activation(out=pt[:], in_=sc[:], func=Exp, scale=0.125)
            # next group's scores first: nothing else may delay the PE
            # work that feeds ACT
            if g + 1 < NTQ * NCH:
                ntqt, nj = divmod(g + 1, NCH)
                sc_next = emit_scores(ntqt, nj)
            # interleaved work (projections, previous tile's out-proj)
            for fn in extra.get(g, ()):
                fn()
            if tqt >= 1 and j in CLOSE_SLOTS and (tqt - 1) in pend:
                pend[tqt - 1][CLOSE_SLOTS.index(j)]()
            for h in range(2):
                MM(
                    pv_tiles[h][:],
                    v_store[j][:, h, :],
                    pt[:, h * 512 : (h + 1) * 512],
                    start=(j == 0),
                    stop=(j == NCH - 1),
                    skip_group_check=True,
                )
            if j == NCH - 1 and tqt < NTQ - 1:
                pvsb = emit_close(tqt, pv_tiles)
                pend[tqt] = make_close_steps(tqt, pvsb)

        # ---- tail: last tile takes a latency-optimized path. The
        # out-projection runs on UNNORMALIZED O right away; the
        # denominators take ONE bounce to spread per-token across
        # partitions, are reciprocal'd there ([128,8], 8 recips worth in
        # one op), and the division is applied afterwards per token row:
        # out = pa*rc0 + pb*rc1 (scalar_tensor_tensor). Filler matmuls
        # keep HAM at K=8/8 through the bounce wait. ----
        tqt = NTQ - 1
        dsb = wsp.tile([1, 1024], f32, tag="denb", name="denb7")
        for h in range(2):
            nc.vector.tensor_copy(
                out=dsb[0:1, h * 512 : (h + 1) * 512], in_=pv_tiles[h][64:65, :]
            )
        o_u7 = []
        for h in range(2):
            ou = opool.tile([DK, 512], bf, tag="oh", name="oh7")
            nc.vector.tensor_copy(out=ou[:], in_=pv_tiles[h][0:64, :])
            o_u7.append(ou)
        dma(out=rden[tqt : tqt + 1, :], in_=dsb[:])
        rc_all = wsp.tile([128, 8], f32, tag="sp", name="rc_all")
        dma(
            out=rc_all[:],
            in_=rden[tqt].rearrange("(h t p) -> p (h t)", h=2, t=4),
        )
        rcr = wsp.tile([128, 8], f32, tag="sp2", name="rcr")
        nc.vector.reciprocal(out=rcr[:], in_=rc_all[:])
        wps2 = projp.tile([128, 512], f32, tag="proj", name="tail_warm")
        for _ in range(8):
            MM(
                wps2[0:1, :],
                ones1[0:1, 0:1],
                ones1[0:1, :],
                start=True,
                stop=True,
                skip_group_check=True,
            )
        Mult = mybir.AluOpType.mult
        Add = mybir.AluOpType.add
        for tt in range(4):
            pa = projp.tile([128, 512], f32, tag="proj", name="pa7")
            MM(
                pa[:],
                o_u7[0][:, tt * 128 : (tt + 1) * 128],
                wo_t[:, 0, :],
                start=True,
                stop=True,
                skip_group_check=True,
            )
            pb = projp.tile([128, 512], f32, tag="proj", name="pb7")
            MM(
                pb[:],
                o_u7[1][:, tt * 128 : (tt + 1) * 128],
                wo_t[:, 1, :],
                start=True,
                stop=True,
                skip_group_check=True,
            )
            t1 = ostage.tile([128, 512], f32, tag="ot", name="ot1")
            nc.vector.scalar_tensor_tensor(
                out=t1[:],
                in0=pb[:],
                scalar=rcr[:, 4 + tt : 5 + tt],
                in1=zeros_t[:],
                op0=Mult,
                op1=Add,
            )
            t2 = ostage.tile([128, 512], bf, tag="ot2", name="ot2")
            nc.vector.scalar_tensor_tensor(
                out=t2[:],
                in0=pa[:],
                scalar=rcr[:, tt : tt + 1],
                in1=t1[:],
                op0=Mult,
                op1=Add,
            )
            dma(
                out=out_p[tqt * 512 + tt * 128 : tqt * 512 + (tt + 1) * 128, :],
                in_=t2[:],
            )

    if not nc.is_finalized():
        nc.finalize()
    return nc


def _get_program():
    global _PROGRAM
    if _PROGRAM is None:
        _PROGRAM = _build_program()
    return _PROGRAM


def _prep_inputs(q, k, v, w_q, b_q, w_k, b_k, w_v, b_v, w_o, b_o):
    bf16 = ml_dtypes.bfloat16
    q = np.asarray(q, dtype=np.float32)
    k = np.asarray(k, dtype=np.float32)
    v = np.asarray(v, dtype=np.float32)
    w_q = np.asarray(w_q, np.float32)
    w_k = np.asarray(w_k, np.float32)
    w_v = np.asarray(w_v, np.float32)
    w_o = np.asarray(w_o, np.float32)
    b_q = np.asarray(b_q, np.float32)
    b_k = np.asarray(b_k, np.float32)
    b_v = np.asarray(b_v, np.float32)
    b_o = np.asarray(b_o, np.float32)

    qT = [np.ascontiguousarray(q[b].T).astype(bf16) for b in range(B)]
    kTb = [np.ascontiguousarray(k[b].T).astype(bf16) for b in range(B)]
    vTb = [np.ascontiguousarray(v[b].T).astype(bf16) for b in range(B)]
    wqT = np.ascontiguousarray(w_q.T).astype(bf16)  # [D_in, D_out]
    wkT = np.ascontiguousarray(w_k.T).astype(bf16)
    wvT = np.ascontiguousarray(w_v.T).astype(bf16)
    woT = np.ascontiguousarray(w_o.T)  # [D_in(head dims), D_out] f32

    in_maps = []
    for c in range(N_CORES):
        b, p = divmod(c, 4)
        ds = slice(p * PD, (p + 1) * PD)
        in_maps.append(
            {
                "qT": qT[b],
                "kT": kTb[b],
                "vT": vTb[b],
                "wqT": np.ascontiguousarray(wqT[:, ds]),
                "wkT": np.ascontiguousarray(wkT[:, ds]),
                "wvT": np.ascontiguousarray(wvT[:, ds]),
                "woT": np.ascontiguousarray(woT[ds, :]).astype(bf16),
                "bq": b_q[ds].reshape(1, PD).astype(bf16),
                "bk": b_k[ds].reshape(1, PD).astype(bf16),
            }
        )
    # V/O biases are exact per-token constants: fold on the host instead
    # of spending device matmuls (partials exclude them; added in combine)
    bias_full = (b_v @ woT + b_o).astype(np.float32)
    return in_maps, bias_full


def run_cores(in_maps, trace=False, **kw):
    """Compile+run the SPMD program; returns BassKernelResults."""
    from concourse.bass_utils import run_bass_kernel_spmd

    nc = _get_program()
    return run_bass_kernel_spmd(nc, in_maps, list(range(N_CORES)), trace=trace, **kw)


def combine_outputs(res, bias_full):
    """Sum the per-core partial outputs into the full [B, S, D] result."""
    out = np.zeros((B, S, D), np.float32)
    for c in range(N_CORES):
        b = c // 4
        out[b] += res.results[c]["out"]
    out += bias_full
    return out


def kernel(q, k, v, w_q, b_q, w_k, b_k, w_v, b_v, w_o, b_o):
    in_maps, bias_full = _prep_inputs(
        q, k, v, w_q, b_q, w_k, b_k, w_v, b_v, w_o, b_o
    )
    res = run_cores(in_maps)
    return combine_outputs(res, bias_full)


# revision 38
# speedup vs baseline: 1.1763x; 1.0019x over previous
"""Multi-head attention (B=2, S=4096, D=512, H=8) on 8 TRN2 NeuronCores.

Sharding: batch x head-pair (tensor parallel). Core c handles batch
b=c//4 and heads {2p, 2p+1} with p=c%4, over the FULL 4096-token
sequence. Q/K/V/O projections are sliced along the head dimension
(each core projects only its 128 dims), eliminating the redundant
full K/V projection of token-sharding. Each core emits a PARTIAL
bf16 output (its heads' contribution through w_o); the host sums the
four partials per batch and adds the V/O bias term (b_v @ w_o.T +
b_o), which is an exact per-token constant. Q/K biases are added
on-device via rank-1 matmuls.

Attention is flash-style with scores kept transposed [tk, tq]. The
two heads' score matmuls (contraction 64) are issued back-to-back at
PE row groups 0/64, so they run CONCURRENTLY in the systolic array
(row tiling) - scores cost half of token-sharding. Softmax skips the
max-subtraction (scores ~ N(0,1)) and the denominator comes from a
ones column appended to V, so softmax is exactly one ACT pass per
score block. The kernel is ACT(exp)-bound: 256 activations of
[128,1024] (sustained ~1.05us each at 1.2 GHz) are the critical
path. Schedule rules discovered on HW: emit next-group scores before
anything else after each exp; never share a PSUM bank between a DVE
read and in-flight PE writes (fatal PSUM collision); keep the
reciprocal on many partitions ([128,8] via a DRAM bounce - a [1,512]
single-lane reciprocal is 3.3us and stalls the DVE queue); warm the
PE through the DMA prefix and the tail bounce so HAM stays at 2.4
GHz.
"""

import numpy as np
import ml_dtypes

B, S, D = 2, 4096, 512
H, DK = 8, 64
N_CORES = 8
PD = 128  # dims per core (2 heads x 64)
NTQ = 8  # tq tiles of 512
NCH = 32  # tk chunks of 128

_PROGRAM = None


def _build_program():
    from contextlib import ExitStack

    import concourse.mybir as mybir
    import concourse.tile as tile
    from concourse import bacc

    bf = mybir.dt.bfloat16
    f32 = mybir.dt.float32
    Exp = mybir.ActivationFunctionType.Exp

    nc = bacc.Bacc(None)

    qT = nc.declare_dram_parameter("qT", [D, S], bf, isOutput=False)
    kT = nc.declare_dram_parameter("kT", [D, S], bf, isOutput=False)
    vT = nc.declare_dram_parameter("vT", [D, S], bf, isOutput=False)
    wqT = nc.declare_dram_parameter("wqT", [D, PD], bf, isOutput=False)
    wkT = nc.declare_dram_parameter("wkT", [D, PD], bf, isOutput=False)
    wvT = nc.declare_dram_parameter("wvT", [D, PD], bf, isOutput=False)
    woT = nc.declare_dram_parameter("woT", [PD, D], bf, isOutput=False)
    bq = nc.declare_dram_parameter("bq", [1, PD], bf, isOutput=False)
    bk = nc.declare_dram_parameter("bk", [1, PD], bf, isOutput=False)
    out_p = nc.declare_dram_parameter("out", [S, D], bf, isOutput=True)
    # DRAM scratch for the denominator-reciprocal partition spread
    rden = nc.dram_tensor("rden", [NTQ, 1024], f32)
    rrec = nc.dram_tensor("rrec", [NTQ, 1024], f32)

    with tile.TileContext(nc) as tc, ExitStack() as ctx:
        wpool = ctx.enter_context(tc.tile_pool(name="w", bufs=1))
        kstream = ctx.enter_context(tc.tile_pool(name="kstream", bufs=2))
        qstream = ctx.enter_context(tc.tile_pool(name="qstream", bufs=2))
        vstream = ctx.enter_context(tc.tile_pool(name="vstream", bufs=2))
        khpool = ctx.enter_context(tc.tile_pool(name="kh", bufs=1))
        qhpool = ctx.enter_context(tc.tile_pool(name="qh", bufs=1))
        vstore = ctx.enter_context(tc.tile_pool(name="vstore", bufs=33))
        ptpool = ctx.enter_context(tc.tile_pool(name="pt", bufs=4))
        opool = ctx.enter_context(tc.tile_pool(name="o", bufs=4))
        wsp = ctx.enter_context(tc.tile_pool(name="ws", bufs=6))
        ostage = ctx.enter_context(tc.tile_pool(name="ostage", bufs=2))
        scorep = ctx.enter_context(tc.tile_pool(name="scorep", bufs=2, space="PSUM"))
        pvp = ctx.enter_context(tc.tile_pool(name="pvp", bufs=2, space="PSUM"))
        projp = ctx.enter_context(tc.tile_pool(name="projp", bufs=2, space="PSUM"))

        dma = nc.sync.dma_start
        MM = nc.tensor.matmul

        # ---- constants (DMA order = sync-queue order = the startup
        # critical path: K-proj inputs first, then Q, then V, the rest) ----
        ones1 = wpool.tile([1, D], bf, tag="ones", name="ones1")
        nc.vector.memset(ones1[:], 1.0)
        # warm the ACT table (exp) during the DMA-heavy prefix
        wrm = wsp.tile([1, 16], f32, tag="denb", name="warm")
        nc.vector.memset(wrm[:], 0.0)
        wrm2 = wsp.tile([1, 16], f32, tag="rrow", name="warm2")
        nc.scalar.activation(out=wrm2[:], in_=wrm[:], func=Exp, scale=1.0)
        # keep the PE busy through the DMA-bound prefix so HAM reaches
        # K=8/8 before (and stays there for) the real matmul stream
        zeros_t = wpool.tile([128, 512], f32, tag="zeros", name="zeros_t")
        nc.vector.memset(zeros_t[:], 0.0)
        wps = projp.tile([128, 512], f32, tag="proj", name="warm_ps")
        for _ in range(10):
            MM(
                wps[0:1, :],
                ones1[0:1, 0:1],
                ones1[0:1, :],
                start=True,
                stop=True,
                skip_group_check=True,
            )

        def wtiles(param, tagp):
            t = wpool.tile([128, 4, PD], bf, tag=tagp, name=tagp)
            dma(out=t[:], in_=param[:].rearrange("(c p) d -> p c d", p=128))
            return t

        khT = khpool.tile([PD, S], bf, tag="khT", name="khT")
        qhT = qhpool.tile([PD, S], bf, tag="qhT", name="qhT")
        v_store = [None] * NCH  # [128 tok, 2 heads, DK+1]; col 64 = ones

        kraw_t = {}
        qraw_t = {}
        vraw_t = {}

        def dma_kraw(t):
            kr = kstream.tile([128, 4, 512], bf, tag="kraw", name="kraw")
            dma(
                out=kr[:],
                in_=kT[:, t * 512 : (t + 1) * 512].rearrange(
                    "(c p) t -> p c t", p=128
                ),
            )
            kraw_t[t] = kr

        def dma_qraw(t):
            qr = qstream.tile([128, 4, 512], bf, tag="qraw", name="qraw")
            dma(
                out=qr[:],
                in_=qT[:, t * 512 : (t + 1) * 512].rearrange(
                    "(c p) t -> p c t", p=128
                ),
            )
            qraw_t[t] = qr

        def dma_vraw(r):
            vr = vstream.tile([128, 4, 512], bf, tag="vraw", name="vraw")
            dma(
                out=vr[:],
                in_=vT[:, r * 512 : (r + 1) * 512].rearrange(
                    "(c p) t -> p c t", p=128
                ),
            )
            vraw_t[r] = vr

        # K/Q projections are split into two closures (2 MMs, then 2 MMs
        # + bias + copy) so no single group carries a >1us PE burst that
        # would starve ACT. The PSUM tile lives across the two groups.
        qk_ps = {}

        def proj_qk_p1(key, raw, w_t):
            ps = projp.tile([128, 512], f32, tag="proj", name="proj_ps")
            qk_ps[key] = ps
            for kk in range(2):
                MM(
                    ps[:],
                    w_t[:, kk, :],
                    raw[:, kk, :],
                    start=(kk == 0),
                    stop=False,
                    skip_group_check=True,
                )

        def proj_qk_p2(key, raw, w_t, b_t, dst, t):
            ps = qk_ps.pop(key)
            for kk in range(2, 4):
                MM(
                    ps[:],
                    w_t[:, kk, :],
                    raw[:, kk, :],
                    start=False,
                    stop=False,
                    skip_group_check=True,
                )
            MM(
                ps[:],
                b_t[:],
                ones1[0:1, 0:512],
                start=False,
                stop=True,
                skip_group_check=True,
            )
            nc.vector.tensor_copy(out=dst[:, t * 512 : (t + 1) * 512], in_=ps[:])

        def proj_qk(raw, w_t, b_t, dst, t):
            proj_qk_p1(("x", t), raw, w_t)
            proj_qk_p2(("x", t), raw, w_t, b_t, dst, t)

        def proj_v_sub(r, sub):
            """Project V tokens (4r+sub)*128.. into v_store[4r+sub].

            Fresh PSUM tile per sub-chunk: sharing one bank across
            sub-chunks makes the DVE copy of chunk n concurrent with PE
            writes of chunk n+1 in the same bank (fatal PSUM collision).
            """
            j = 4 * r + sub
            ps = projp.tile([128, 512], f32, tag="proj", name="vps")
            for kk in range(4):
                MM(
                    ps[:, 0:128],
                    vraw_t[r][:, kk, sub * 128 : (sub + 1) * 128],
                    wv_t[:, kk, :],
                    start=(kk == 0),
                    stop=(kk == 3),
                    skip_group_check=True,
                )
            vs = vstore.tile([128, 2, DK + 1], bf, tag="vs", name="vs")
            v_store[j] = vs
            nc.vector.memset(vs[:, :, DK : DK + 1], 1.0)
            nc.vector.tensor_copy(
                out=vs[:, :, 0:DK],
                in_=ps[:, 0:128].rearrange("p (h c) -> p h c", c=DK),
            )

        def emit_scores(tqt, j):
            sc = scorep.tile([128, 1024], f32, tag="sc", name="sc")
            for h in range(2):
                pb = h * 64
                MM(
                    sc[:, h * 512 : (h + 1) * 512],
                    khT[pb : pb + 64, j * 128 : (j + 1) * 128],
                    qhT[pb : pb + 64, tqt * 512 : (tqt + 1) * 512],
                    start=True,
                    stop=True,
                    skip_group_check=True,
                )
            return sc

        # ---- closure schedule: group index -> list of closures ----
        extra = {}

        def add(g, fn):
            extra.setdefault(g, []).append(fn)

        # K tiles 1-7: dma 4 groups ahead; matmuls split over two groups
        for t in range(1, 8):
            add(4 * t - 4, lambda t=t: dma_kraw(t))
            add(4 * t - 3, lambda t=t: proj_qk_p1(("k", t), kraw_t[t], wk_t))
            add(
                4 * t - 2,
                lambda t=t: proj_qk_p2(
                    ("k", t), kraw_t[t], wk_t, bk_t, khT, t
                ),
            )
        # V raw streams r=2..7 (r=0,1 in prefix); chunk j's projection at
        # group j-1 (one per group - its MMs precede PV(j-1), and PV(j)
        # only runs a full group later)
        for r in range(2, 8):
            add(4 * r - 6, lambda r=r: dma_vraw(r))
        for j in range(1, NCH):
            add(j - 1, lambda r=j // 4, s=j % 4: proj_v_sub(r, s))
        # Q tiles 1-7 projected near the end of the previous tq tile,
        # split over groups 26/27 (NOT 25/26: at tile-0 group 25 the K7
        # projection tile is live until its group-26 copy, and a third
        # concurrent projp tile there creates a PE<->DVE emission-order
        # dependency cycle through the 2-deep ring)
        for T in range(1, 8):
            add((T - 1) * 32 + 24, lambda T=T: dma_qraw(T))
            add(
                (T - 1) * 32 + 26,
                lambda T=T: proj_qk_p1(("q", T), qraw_t[T], wq_t),
            )
            add(
                (T - 1) * 32 + 27,
                lambda T=T: proj_qk_p2(
                    ("q", T), qraw_t[T], wq_t, bq_t, qhT, T
                ),
            )

        # ---- tile close + normalization/out-projection ----
        # At tile close: copy both PV accumulators to SBUF (frees the
        # PSUM banks for the next tile), pack the two denominator rows
        # into one [2,512] tile and bounce it through DRAM to spread it
        # as [128,8] (h*64+p lanes) so the reciprocal runs 8 elems/lane.
        # One bounce for both heads keeps the sync queue light.
        def emit_close(tqt, pv_tiles):
            pvsb = []
            for h in range(2):
                t = wsp.tile([DK + 1, 512], f32, tag="pvsb", name="pvsb")
                nc.vector.tensor_copy(out=t[:], in_=pv_tiles[h][:])
                pvsb.append(t)
            dsb = wsp.tile([1, 1024], f32, tag="denb", name="denb")
            for h in range(2):
                nc.vector.tensor_copy(
                    out=dsb[0:1, h * 512 : (h + 1) * 512], in_=pvsb[h][64:65, :]
                )
            dma(out=rden[tqt : tqt + 1, :], in_=dsb[:])
            sp = wsp.tile([128, 8], f32, tag="sp", name="sp")
            dma(out=sp[:], in_=rden[tqt].rearrange("(p e) -> p e", p=128))
            sp2 = wsp.tile([128, 8], f32, tag="sp2", name="sp2")
            nc.vector.reciprocal(out=sp2[:], in_=sp[:])
            dma(out=rrec[tqt].rearrange("(p e) -> p e", p=128), in_=sp2[:])
            return pvsb

        def make_close_steps(tqt, pvsb):
            o_n = [None, None]
            steps = []
            for h in range(2):

                def s2(h=h):
                    w = wsp.tile([64, 512], f32, tag="ws", name="wst")
                    dma(
                        out=w[:],
                        in_=rrec[
                            tqt : tqt + 1, h * 512 : (h + 1) * 512
                        ].partition_broadcast(64),
                    )
                    on = opool.tile([DK, 512], bf, tag="oh", name="oh")
                    nc.vector.tensor_mul(out=on[:], in0=pvsb[h][0:64, :], in1=w[:])
                    o_n[h] = on

                steps.append(s2)
            for tt in range(4):

                def s3(tt=tt):
                    pa = projp.tile([128, 512], f32, tag="proj", name="pa")
                    for h in range(2):
                        MM(
                            pa[:],
                            o_n[h][:, tt * 128 : (tt + 1) * 128],
                            wo_t[:, h, :],
                            start=(h == 0),
                            stop=(h == 1),
                            skip_group_check=True,
                        )
                    ot = ostage.tile([128, 512], bf, tag="ot", name="ot")
                    nc.vector.tensor_copy(out=ot[:], in_=pa[:])
                    dma(
                        out=out_p[
                            tqt * 512 + tt * 128 : tqt * 512 + (tt + 1) * 128, :
                        ],
                        in_=ot[:],
                    )

                steps.append(s3)
            return steps

        CLOSE_SLOTS = (3, 4, 7, 11, 15, 19)
        pend = {}

        # ---- prefix: interleave input DMAs with the projections that
        # consume them so the first exp fires as early as possible ----
        dma_kraw(0)
        wk_t = wtiles(wkT, "wk")
        bk_t = wpool.tile([1, PD], bf, tag="bk", name="bk_t")
        dma(out=bk_t[:], in_=bk[:])
        dma_qraw(0)
        wq_t = wtiles(wqT, "wq")
        bq_t = wpool.tile([1, PD], bf, tag="bq", name="bq_t")
        dma(out=bq_t[:], in_=bq[:])
        dma_vraw(0)
        wv_t = wtiles(wvT, "wv")
        # [64, 2, D]: wo_t[:, h, :] puts both heads' w_o rows at base
        # partition 0, so out-proj MMs share row group 0 (concurrent
        # row-group accumulation into one PSUM bank races).
        wo_t = wpool.tile([DK, 2, D], bf, tag="wo", name="wo_t")
        dma(out=wo_t[:], in_=woT[:].rearrange("(h p) d -> p h d", p=DK))
        dma_vraw(1)
        proj_qk(kraw_t[0], wk_t, bk_t, khT, 0)
        proj_qk(qraw_t[0], wq_t, bq_t, qhT, 0)

        # ---- main attention loop ----
        sc_next = emit_scores(0, 0)
        # V chunk 0 projected after the first scores: exp(0) must not
        # queue behind V matmuls, and PV(0,0) only needs it post-exp
        proj_v_sub(0, 0)
        pv_tiles = None
        for g in range(NTQ * NCH):
            tqt, j = divmod(g, NCH)
            if j == 0:
                pv_tiles = [
                    pvp.tile([DK + 1, 512], f32, tag="pv", name=f"pv{_h}")
                    for _h in range(2)
                ]
            sc = sc_next
            pt = ptpool.tile([128, 1024], bf, tag="pt", name="pt")
            nc.scalar.activation(out=pt[:], in_=sc[:], func=Exp, scale=0.125)
            # next group's scores first: nothing else may delay the PE
            # work that feeds ACT
            if g + 1 < NTQ * NCH:
                ntqt, nj = divmod(g + 1, NCH)
                sc_next = emit_scores(ntqt, nj)
            # interleaved work (projections, previous tile's out-proj)
            for fn in extra.get(g, ()):
                fn()
            if tqt >= 1 and j in CLOSE_SLOTS and (tqt - 1) in pend:
                pend[tqt - 1][CLOSE_SLOTS.index(j)]()
            for h in range(2):
                MM(
                    pv_tiles[h][:],
                    v_store[j][:, h, :],
                    pt[:, h * 512 : (h + 1) * 512],
                    start=(j == 0),
                    stop=(j == NCH - 1),
                    skip_group_check=True,
                )
            if j == NCH - 1 and tqt < NTQ - 1:
                pvsb = emit_close(tqt, pv_tiles)
                pend[tqt] = make_close_steps(tqt, pvsb)

        # ---- tail: last tile takes a latency-optimized path. The
        # out-projection runs on UNNORMALIZED O right away; the
        # denominators take ONE bounce to spread per-token across
        # partitions, are reciprocal'd there ([128,8], 8 recips worth in
        # one op), and the division is applied afterwards per token row:
        # out = pa*rc0 + pb*rc1 (scalar_tensor_tensor). Filler matmuls
        # keep HAM at K=8/8 through the bounce wait. ----
        tqt = NTQ - 1
        dsb = wsp.tile([1, 1024], f32, tag="denb", name="denb7")
        for h in range(2):
            nc.vector.tensor_copy(
                out=dsb[0:1, h * 512 : (h + 1) * 512], in_=pv_tiles[h][64:65, :]
            )
        o_u7 = []
        for h in range(2):
            ou = opool.tile([DK, 512], bf, tag="oh", name="oh7")
            nc.vector.tensor_copy(out=ou[:], in_=pv_tiles[h][0:64, :])
            o_u7.append(ou)
        dma(out=rden[tqt : tqt + 1, :], in_=dsb[:])
        rc_all = wsp.tile([128, 8], f32, tag="sp", name="rc_all")
        dma(
            out=rc_all[:],
            in_=rden[tqt].rearrange("(h t p) -> p (h t)", h=2, t=4),
        )
        rcr = wsp.tile([128, 8], f32, tag="sp2", name="rcr")
        nc.vector.reciprocal(out=rcr[:], in_=rc_all[:])
        wps2 = projp.tile([128, 512], f32, tag="proj", name="tail_warm")
        for _ in range(8):
            MM(
                wps2[0:1, :],
                ones1[0:1, 0:1],
                ones1[0:1, :],
                start=True,
                stop=True,
                skip_group_check=True,
            )
        Mult = mybir.AluOpType.mult
        Add = mybir.AluOpType.add
        for tt in range(4):
            pa = projp.tile([128, 512], f32, tag="proj", name="pa7")
            MM(
                pa[:],
                o_u7[0][:, tt * 128 : (tt + 1) * 128],
                wo_t[:, 0, :],
                start=True,
                stop=True,
                skip_group_check=True,
            )
            pb = projp.tile([128, 512], f32, tag="proj", name="pb7")
            MM(
                pb[:],
                o_u7[1][:, tt * 128 : (tt + 1) * 128],
                wo_t[:, 1, :],
                start=True,
                stop=True,
                skip_group_check=True,
            )
            t1 = ostage.tile([128, 512], f32, tag="ot", name="ot1")
            nc.vector.scalar_tensor_tensor(
                out=t1[:],
                in0=pb[:],
                scalar=rcr[:, 4 + tt : 5 + tt],
                in1=zeros_t[:],
                op0=Mult,
                op1=Add,
            )
            t2 = ostage.tile([128, 512], bf, tag="ot2", name="ot2")
            nc.vector.scalar_tensor_tensor(
                out=t2[:],
                in0=pa[:],
                scalar=rcr[:, tt : tt + 1],
                in1=t1[:],
                op0=Mult,
                op1=Add,
            )
            dma(
                out=out_p[tqt * 512 + tt * 128 : tqt * 512 + (tt + 1) * 128, :],
                in_=t2[:],
            )

    if not nc.is_finalized():
        nc.finalize()
    return nc


def _get_program():
    global _PROGRAM
    if _PROGRAM is None:
        _PROGRAM = _build_program()
    return _PROGRAM


def _prep_inputs(q, k, v, w_q, b_q, w_k, b_k, w_v, b_v, w_o, b_o):
    bf16 = ml_dtypes.bfloat16
    q = np.asarray(q, dtype=np.float32)
    k = np.asarray(k, dtype=np.float32)
    v = np.asarray(v, dtype=np.float32)
    w_q = np.asarray(w_q, np.float32)
    w_k = np.asarray(w_k, np.float32)
    w_v = np.asarray(w_v, np.float32)
    w_o = np.asarray(w_o, np.float32)
    b_q = np.asarray(b_q, np.float32)
    b_k = np.asarray(b_k, np.float32)
    b_v = np.asarray(b_v, np.float32)
    b_o = np.asarray(b_o, np.float32)

    qT = [np.ascontiguousarray(q[b].T).astype(bf16) for b in range(B)]
    kTb = [np.ascontiguousarray(k[b].T).astype(bf16) for b in range(B)]
    vTb = [np.ascontiguousarray(v[b].T).astype(bf16) for b in range(B)]
    wqT = np.ascontiguousarray(w_q.T).astype(bf16)  # [D_in, D_out]
    wkT = np.ascontiguousarray(w_k.T).astype(bf16)
    wvT = np.ascontiguousarray(w_v.T).astype(bf16)
    woT = np.ascontiguousarray(w_o.T)  # [D_in(head dims), D_out] f32

    in_maps = []
    for c in range(N_CORES):
        b, p = divmod(c, 4)
        ds = slice(p * PD, (p + 1) * PD)
        in_maps.append(
            {
                "qT": qT[b],
                "kT": kTb[b],
                "vT": vTb[b],
                "wqT": np.ascontiguousarray(wqT[:, ds]),
                "wkT": np.ascontiguousarray(wkT[:, ds]),
                "wvT": np.ascontiguousarray(wvT[:, ds]),
                "woT": np.ascontiguousarray(woT[ds, :]).astype(bf16),
                "bq": b_q[ds].reshape(1, PD).astype(bf16),
                "bk": b_k[ds].reshape(1, PD).astype(bf16),
            }
        )
    # V/O biases are exact per-token constants: fold on the host instead
    # of spending device matmuls (partials exclude them; added in combine)
    bias_full = (b_v @ woT + b_o).astype(np.float32)
    return in_maps, bias_full


def run_cores(in_maps, trace=False, **kw):
    """Compile+run the SPMD program; returns BassKernelResults."""
    from concourse.bass_utils import run_bass_kernel_spmd

    nc = _get_program()
    return run_bass_kernel_spmd(nc, in_maps, list(range(N_CORES)), trace=trace, **kw)


def combine_outputs(res, bias_full):
    """Sum the per-core partial outputs into the full [B, S, D] result."""
    out = np.zeros((B, S, D), np.float32)
    for c in range(N_CORES):
        b = c // 4
        out[b] += res.results[c]["out"]
    out += bias_full
    return out


def kernel(q, k, v, w_q, b_q, w_k, b_k, w_v, b_v, w_o, b_o):
    in_maps, bias_full = _prep_inputs(
        q, k, v, w_q, b_q, w_k, b_k, w_v, b_v, w_o, b_o
    )
    res = run_cores(in_maps)
    return combine_outputs(res, bias_full)
